# revision 1
# baseline (speedup 1.0000x reference)
"""Trainium2 Bass kernel for causal MHA (B=2, T=2048, D=1024, H=16, KH=64).

Sharding: 8 cores = 2 (batch) x 4 (head groups of 4 heads).
Each core computes q/k/v projections for its 4 heads, causal attention,
and a partial output projection against its 256-row slice of Wout.
Host sums the 4 partials per batch (the all-reduce step, done at unshard).
"""
import sys

sys.path.insert(0, "/opt/trn_rl_repo")

from contextlib import ExitStack

import numpy as np

import concourse.bacc as bacc
import concourse.mybir as mybir
import concourse.tile as tile

B, T, C = 2, 2048, 1024
H, KH = 16, 64
G = 4                 # head groups
HPG = H // G          # heads per group = 4
DG = HPG * KH         # 256 per-core head dims
NCORES = 8

F32 = mybir.dt.float32
F32R = mybir.dt.float32r
EXP = mybir.ActivationFunctionType.Exp
COPY = mybir.ActivationFunctionType.Copy

_cached_nc = None


def build_nc(phases=3):
    nc = bacc.Bacc()
    xt = nc.dram_tensor("xt", [C, T], F32R, kind="ExternalInput")        # x[b].T
    wq = nc.dram_tensor("wq", [C, DG], F32R, kind="ExternalInput")       # Wq slice .T
    wk = nc.dram_tensor("wk", [C, DG], F32R, kind="ExternalInput")
    wv = nc.dram_tensor("wv", [C, DG], F32R, kind="ExternalInput")
    wo = nc.dram_tensor("wo", [DG, C], F32R, kind="ExternalInput")       # Wout[:, slice].T
    keep = nc.dram_tensor("keep", [128, T], F32R, kind="ExternalInput")  # diag keep blocks (k, q)
    y = nc.dram_tensor("y", [T, C], F32, kind="ExternalOutput")          # partial output

    NT = T // 512     # 4 moving t tiles
    NK = C // 128     # 8 contraction chunks
    NTT = T // 128    # 16 t tiles of 128

    with ExitStack() as ctx:
        ctx.enter_context(nc.allow_low_precision(reason="f32r matmul pipeline"))
        tc = ctx.enter_context(tile.TileContext(nc))
        persist = ctx.enter_context(tc.tile_pool(name="persist", bufs=1))
        psum = ctx.enter_context(tc.tile_pool(name="psum", bufs=2, space="PSUM"))

        # ---- persistent tiles ----
        qT = [persist.tile([128, T], F32R, tag=f"qT{i}", name=f"qT{i}") for i in range(2)]
        kT = [persist.tile([128, T], F32R, tag=f"kT{i}", name=f"kT{i}") for i in range(2)]
        vsb = [persist.tile([128, HPG, KH + 1], F32R, tag=f"v{i}", name=f"v{i}")
               for i in range(NTT)]
        aTn = [persist.tile([64, T], F32R, tag=f"aTn{h}", name=f"aTn{h}")
               for h in range(HPG)]
        wo_sb = [persist.tile([64, C], F32R, tag=f"wo{h}", name=f"wo{h}")
                 for h in range(HPG)]
        keep_sb = persist.tile([128, T], F32R, tag="keep")
        ones_sb = persist.tile([65, 64], F32R, tag="ones")
        ones_f32 = persist.tile([65, 64], F32, tag="ones_f32")
        onecol_f32 = persist.tile([128, HPG, 1], F32, tag="onecol_f32")

        nc.sync.dma_start(out=keep_sb, in_=keep[:, :])
        for h in range(HPG):
            nc.sync.dma_start(out=wo_sb[h], in_=wo[h * KH:(h + 1) * KH, :])
        nc.vector.memset(ones_f32, 1.0)
        nc.vector.tensor_copy(out=ones_sb, in_=ones_f32)
        nc.vector.memset(onecol_f32, 1.0)

        # ================= Phase 1: projections =================
        with tc.tile_pool(name="ph1", bufs=1) as ph1:
            xT = [ph1.tile([128, T], F32R, tag=f"xT{k}", name=f"xT{k}")
                  for k in range(NK)]
            wq_sb = [ph1.tile([128, DG], F32R, tag=f"wq{k}", name=f"wq{k}")
                     for k in range(NK)]
            wk_sb = [ph1.tile([128, DG], F32R, tag=f"wk{k}", name=f"wk{k}")
                     for k in range(NK)]
            wv_sb = [ph1.tile([128, DG], F32R, tag=f"wv{k}", name=f"wv{k}")
                     for k in range(NK)]
            for k in range(NK):
                nc.sync.dma_start(out=xT[k], in_=xt[k * 128:(k + 1) * 128, :])
                nc.sync.dma_start(out=wq_sb[k], in_=wq[k * 128:(k + 1) * 128, :])
                nc.sync.dma_start(out=wk_sb[k], in_=wk[k * 128:(k + 1) * 128, :])
                nc.sync.dma_start(out=wv_sb[k], in_=wv[k * 128:(k + 1) * 128, :])

            # qT/kT: (dk 128-pair, t) = sum_c w[c, dk].T . xT[c, t]
            pi = 0
            for dst, w_sb in ((qT, wq_sb), (kT, wk_sb)):
                for m in range(2):          # head pair -> partition block
                    for n in range(NT):     # moving t tile of 512
                        ps = psum.tile([128, 512], F32, tag=f"ps{pi % 2}", name="ps")
                        pi += 1
                        for k in range(NK):
                            nc.tensor.matmul(
                                ps,
                                w_sb[k][:, m * 128:(m + 1) * 128],
                                xT[k][:, n * 512:(n + 1) * 512],
                                start=(k == 0), stop=(k == NK - 1),
                            )
                        nc.scalar.activation(
                            out=dst[m][:, n * 512:(n + 1) * 512], in_=ps, func=COPY)
            # V: (t 128, dv 256) = sum_c xT[c, t].T . wv[c, dv]  (+ ones col)
            for tt in range(NTT):
                ps = psum.tile([128, DG], F32, tag=f"ps{pi % 2}", name="ps")
                pi += 1
                for k in range(NK):
                    nc.tensor.matmul(
                        ps,
                        xT[k][:, tt * 128:(tt + 1) * 128],
                        wv_sb[k],
                        start=(k == 0), stop=(k == NK - 1),
                    )
                nc.vector.tensor_copy(
                    out=vsb[tt][:, :, 0:KH],
                    in_=ps[:].rearrange("p (h d) -> p h d", h=HPG),
                )
                nc.vector.tensor_copy(out=vsb[tt][:, :, KH:KH + 1], in_=onecol_f32)

        if phases < 2:
            nc.sync.dma_start(out=y[0:128, :], in_=qT[0][:, 0:C].bitcast(F32))
            nc.sync.dma_start(out=y[128:256, :], in_=kT[0][:, 0:C].bitcast(F32))

        # ================= Phase 2: attention =================
        with tc.tile_pool(name="pts", bufs=4) as ptp, \
             tc.tile_pool(name="srowp", bufs=4) as srp:
            for hp in range(2 if phases >= 2 else 0):
                for qj in range(NT):
                    kmax = 4 * qj + 4
                    acc = [psum.tile([65, 512], F32, tag=f"pv{par}", name=f"pv{par}")
                           for par in range(2)]
                    for kt in range(kmax):
                        off = 128 * (kt - 4 * qj) if kt >= 4 * qj else 0
                        for par in range(2):
                            h = 2 * hp + par
                            sc = psum.tile([128, 512 - off], F32, tag=f"ps{par}",
                                           name="sc")
                            nc.tensor.matmul(
                                sc,
                                kT[hp][64 * par:64 * par + 64, kt * 128:(kt + 1) * 128],
                                qT[hp][64 * par:64 * par + 64,
                                       qj * 512 + off:(qj + 1) * 512],
                                start=True, stop=True,
                            )
                            pt = ptp.tile([128, 512], F32R, tag=f"pt{par}", name="pt")
                            nc.scalar.activation(
                                out=pt[:, off:512], in_=sc, func=EXP, scale=0.125)
                            if kt >= 4 * qj:
                                nc.vector.tensor_mul(
                                    pt[:, off:off + 128],
                                    pt[:, off:off + 128],
                                    keep_sb[:, kt * 128:(kt + 1) * 128],
                                )
                            nc.tensor.matmul(
                                acc[par][:, off:512],
                                vsb[kt][:, h, :],
                                pt[:, off:512],
                                start=(kt == 0), stop=(kt == kmax - 1),
                            )
                    for par in range(2):
                        h = 2 * hp + par
                        srow = srp.tile([65, 512], F32R, tag="srow", name="srow")
                        nc.vector.reciprocal(out=srow[64:65, :],
                                             in_=acc[par][64:65, :])
                        nc.vector.tensor_copy(
                            out=aTn[h][:, qj * 512:(qj + 1) * 512],
                            in_=acc[par][0:64, :],
                        )
                        rbc = psum.tile([64, 512], F32, tag=f"ps{par}", name="rbc")
                        nc.tensor.matmul(
                            rbc, ones_sb[64:65, :], srow[64:65, :],
                            start=True, stop=True)
                        nc.vector.tensor_mul(
                            aTn[h][:, qj * 512:(qj + 1) * 512],
                            aTn[h][:, qj * 512:(qj + 1) * 512],
                            rbc,
                        )

        if phases == 2:
            for h in range(HPG):
                nc.sync.dma_start(out=y[h * 128:h * 128 + 64, 0:T // 2],
                                  in_=aTn[h][:, 0:T // 2].bitcast(F32))

        # ================= Phase 3: output projection =================
        with tc.tile_pool(name="ph3", bufs=2) as ph3:
            for tt in range(NTT if phases >= 3 else 0):
                yt = ph3.tile([128, C], F32, tag="ysb", name="yt")
                for no in range(2):
                    yp = psum.tile([128, 512], F32, tag=f"ps{no}", name="yp")
                    for h in range(HPG):
                        nc.tensor.matmul(
                            yp,
                            aTn[h][:, tt * 128:(tt + 1) * 128],
                            wo_sb[h][:, no * 512:(no + 1) * 512],
                            start=(h == 0), stop=(h == HPG - 1),
                        )
                    nc.vector.tensor_copy(out=yt[:, no * 512:(no + 1) * 512], in_=yp)
                nc.sync.dma_start(out=y[tt * 128:(tt + 1) * 128, :], in_=yt)

    _split_excess_waits(nc)
    nc.compile()
    return nc



def _split_excess_waits(nc):
    """Walrus caps most instructions at 1 sync wait. Peel excess waits off
    matmuls (and anything else over the cap) onto PE-engine wait-nops
    inserted immediately before the instruction."""
    for bb in nc.main_func.blocks:
        new_insts = []
        for inst in bb.instructions:
            si = inst.sync_info
            if (si is not None and si.on_wait and len(si.on_wait) > 1
                    and isinstance(inst, mybir.InstMatmult)):
                excess = list(si.on_wait[:-1])
                keep = [si.on_wait[-1]]
                for w in excess:
                    nop = mybir.InstNoOp(
                        name=nc.get_next_instruction_name(), ins=[], outs=[],
                        bass_nofuse=True)
                    nop.engine = inst.engine
                    nop.sync_info = mybir.SyncInfo(on_wait=[w], on_update=[])
                    nc.register_instruction(nop)
                    new_insts.append(nop)
                si.on_wait = keep
            new_insts.append(inst)
        bb.instructions[:] = new_insts


def _host_prep(x, Wq, Wkv, Wout, mask):
    x = np.asarray(x, dtype=np.float32)
    Wq = np.asarray(Wq, dtype=np.float32)
    Wkv = np.asarray(Wkv, dtype=np.float32)
    Wout = np.asarray(Wout, dtype=np.float32)
    mask = np.asarray(mask)

    xT = [np.ascontiguousarray(x[b].T) for b in range(B)]
    keep = np.empty((128, T), dtype=np.float32)
    for i in range(T // 128):
        blk = mask[128 * i:128 * (i + 1), 128 * i:128 * (i + 1)]
        keep[:, 128 * i:128 * (i + 1)] = (~blk).T.astype(np.float32)

    in_maps = []
    for core in range(NCORES):
        b, g = core // G, core % G
        sl = slice(DG * g, DG * (g + 1))
        in_maps.append({
            "xt": xT[b],
            "wq": np.ascontiguousarray(Wq[sl, :].T),
            "wk": np.ascontiguousarray(Wkv[sl, :].T),
            "wv": np.ascontiguousarray(Wkv[C + DG * g:C + DG * (g + 1), :].T),
            "wo": np.ascontiguousarray(Wout[:, sl].T),
            "keep": keep,
        })
    return in_maps


def _install_ntff_hook():
    import types
    import antenv
    if getattr(antenv, "axon_hooks", None) is not None:
        return
    ah = types.ModuleType("antenv.axon_hooks")
    ah._hook = None
    ah.set_axon_ntff_profile_hook = lambda h: setattr(ah, "_hook", h)
    ah.get_axon_ntff_profile_hook = lambda: ah._hook
    sys.modules["antenv.axon_hooks"] = ah
    antenv.axon_hooks = ah
    if "/root/.axon_site" not in sys.path:
        sys.path.insert(0, "/root/.axon_site")
    from trn_agent_boot.trn_boot import _ntff_profile_via_ctypes
    ah.set_axon_ntff_profile_hook(_ntff_profile_via_ctypes("/opt/axon/libaxon_pjrt.so"))


def _run(inputs, trace=False, phases=3):
    global _cached_nc
    from concourse.bass_utils import run_bass_kernel_spmd
    if trace:
        _install_ntff_hook()
    if _cached_nc is None:
        _cached_nc = build_nc(phases=phases)
    in_maps = _host_prep(**inputs)
    res = run_bass_kernel_spmd(_cached_nc, in_maps, list(range(NCORES)), trace=trace)
    parts = [res.results[c]["y"] for c in range(NCORES)]
    out = np.stack([
        parts[0] + parts[1] + parts[2] + parts[3],
        parts[4] + parts[5] + parts[6] + parts[7],
    ]).astype(np.float32)
    return out, res


def kernel(x, Wq, Wkv, Wout, mask):
    out, _ = _run(dict(x=x, Wq=Wq, Wkv=Wkv, Wout=Wout, mask=mask))
    return out



# revision 2
# speedup vs baseline: 1.5324x; 1.5324x over previous
"""Trainium2 Bass kernel for causal MHA (B=2, T=2048, D=1024, H=16, KH=64).

Sharding: 8 cores = 2 (batch) x 4 (head groups of 4 heads).
Each core computes q/k/v projections for its 4 heads, causal attention,
and a partial output projection against its 256-row slice of Wout.
Host sums the 4 partials per batch (the all-reduce step, done at unshard).

v2: bf16 matmul pipeline (fp32r streams trip the power throttler and pin
the PE at 1.2 GHz), fast approximate reciprocal on the broadcast
denominator, pair-batched EXP activations, and output projection
interleaved into the attention loop.
"""
import sys

sys.path.insert(0, "/opt/trn_rl_repo")

from contextlib import ExitStack

import numpy as np
import ml_dtypes

import concourse.bacc as bacc
import concourse.mybir as mybir
import concourse.tile as tile

B, T, C = 2, 2048, 1024
H, KH = 16, 64
G = 4                 # head groups
HPG = H // G          # heads per group = 4
DG = HPG * KH         # 256 per-core head dims
NCORES = 8

F32 = mybir.dt.float32
F32R = mybir.dt.float32r
BF16 = mybir.dt.bfloat16
EXP = mybir.ActivationFunctionType.Exp
COPY = mybir.ActivationFunctionType.Copy

_cached_nc = None


def build_nc():
    nc = bacc.Bacc()
    xt = nc.dram_tensor("xt", [C, T], BF16, kind="ExternalInput")        # x[b].T
    wq = nc.dram_tensor("wq", [C, DG], BF16, kind="ExternalInput")       # Wq slice .T
    wk = nc.dram_tensor("wk", [C, DG], BF16, kind="ExternalInput")
    wv = nc.dram_tensor("wv", [C, DG], BF16, kind="ExternalInput")
    wo = nc.dram_tensor("wo", [DG, C], BF16, kind="ExternalInput")       # Wout[:, slice].T
    keep = nc.dram_tensor("keep", [128, T], BF16, kind="ExternalInput")  # diag keep (k, q)
    y = nc.dram_tensor("y", [T, C], BF16, kind="ExternalOutput")         # partial output

    NT = T // 512     # 4 moving t tiles
    NK = C // 128     # 8 contraction chunks
    NTT = T // 128    # 16 t tiles of 128

    with ExitStack() as ctx:
        ctx.enter_context(nc.allow_low_precision(reason="bf16 matmul pipeline"))
        tc = ctx.enter_context(tile.TileContext(nc))
        persist = ctx.enter_context(tc.tile_pool(name="persist", bufs=1))
        psum = ctx.enter_context(tc.tile_pool(name="psum", bufs=2, space="PSUM"))

        # ---- persistent tiles ----
        qT = [persist.tile([128, T], BF16, tag=f"qT{i}", name=f"qT{i}") for i in range(2)]
        kT = [persist.tile([128, T], BF16, tag=f"kT{i}", name=f"kT{i}") for i in range(2)]
        vsb = [persist.tile([128, HPG, KH + 1], BF16, tag=f"v{i}", name=f"v{i}")
               for i in range(NTT)]
        aTn = [persist.tile([64, T], BF16, tag=f"aTn{h}", name=f"aTn{h}")
               for h in range(HPG)]
        wo_sb = [persist.tile([64, C], BF16, tag=f"wo{h}", name=f"wo{h}")
                 for h in range(HPG)]
        keep_sb = persist.tile([128, T], BF16, tag="keep")
        ones_r = persist.tile([65, 64], F32R, tag="ones_r")
        ones_f32 = persist.tile([65, 64], F32, tag="ones_f32")

        # weights + mask first (small, unblock first matmuls)
        for h in range(HPG):
            nc.sync.dma_start(out=wo_sb[h], in_=wo[h * KH:(h + 1) * KH, :])
        nc.sync.dma_start(out=keep_sb, in_=keep[:, :])
        nc.vector.memset(ones_f32, 1.0)
        nc.vector.tensor_copy(out=ones_r, in_=ones_f32)

        # ================= Phase 1: projections =================
        with tc.tile_pool(name="ph1", bufs=1) as ph1:
            xT = ph1.tile([128, NK, T], BF16, tag="xT", name="xT")
            wq_sb = ph1.tile([128, NK, DG], BF16, tag="wq_sb", name="wq_sb")
            wk_sb = ph1.tile([128, NK, DG], BF16, tag="wk_sb", name="wk_sb")
            wv_sb = ph1.tile([128, NK, DG], BF16, tag="wv_sb", name="wv_sb")
            for w_sb, w_dram in ((wq_sb, wq), (wk_sb, wk), (wv_sb, wv)):
                nc.sync.dma_start(
                    out=w_sb, in_=w_dram.rearrange("(k p) d -> p k d", p=128))
            xt_r = xt.rearrange("(k p) t -> p k t", p=128)
            for n in range(NT):
                nc.sync.dma_start(out=xT[:, :, n * 512:(n + 1) * 512],
                                  in_=xt_r[:, :, n * 512:(n + 1) * 512])

            # qT/kT: (dk 128-block, t) = sum_c w[c, dk].T . xT[c, t]
            for n in range(NT):           # moving t tile of 512
                for dst, w_sb in ((qT, wq_sb), (kT, wk_sb)):
                    for m in range(2):    # head pair -> partition block
                        ps = psum.tile([128, 512], F32, tag="sc", name="ps")
                        for k in range(NK):
                            nc.tensor.matmul(
                                ps,
                                w_sb[:, k, m * 128:(m + 1) * 128],
                                xT[:, k, n * 512:(n + 1) * 512],
                                start=(k == 0), stop=(k == NK - 1),
                            )
                        nc.scalar.activation(
                            out=dst[m][:, n * 512:(n + 1) * 512], in_=ps, func=COPY)
                # V: (t 128, dv 256) = sum_c xT[c, t].T . wv[c, dv]
                for i in range(4):
                    tt = 4 * n + i
                    ps = psum.tile([128, DG], F32, tag="sc", name="ps")
                    for k in range(NK):
                        nc.tensor.matmul(
                            ps,
                            xT[:, k, tt * 128:(tt + 1) * 128],
                            wv_sb[:, k, :],
                            start=(k == 0), stop=(k == NK - 1),
                        )
                    nc.vector.tensor_copy(
                        out=vsb[tt][:, :, 0:KH],
                        in_=ps[:].rearrange("p (h d) -> p h d", h=HPG),
                    )
                    nc.gpsimd.memset(vsb[tt][:, :, KH:KH + 1], 1.0)

        # ============ Phase 2+3: attention, outproj interleaved ============
        def outproj(qj):
            for i in range(4):
                tt = 4 * qj + i
                yt = ytp.tile([128, C], BF16, tag="yt", name="yt")
                for no in range(2):
                    yp = psum.tile([128, 512], F32, tag="sc", name="yp")
                    for h in range(HPG):
                        nc.tensor.matmul(
                            yp,
                            aTn[h][:, tt * 128:(tt + 1) * 128],
                            wo_sb[h][:, no * 512:(no + 1) * 512],
                            start=(h == 0), stop=(h == HPG - 1),
                        )
                    nc.vector.tensor_copy(out=yt[:, no * 512:(no + 1) * 512], in_=yp)
                nc.sync.dma_start(out=y[tt * 128:(tt + 1) * 128, :], in_=yt)

        with tc.tile_pool(name="pts", bufs=4) as ptp, \
             tc.tile_pool(name="srowp", bufs=4) as srp, \
             tc.tile_pool(name="rinvp", bufs=2) as rip, \
             tc.tile_pool(name="ytp", bufs=2) as ytp:
            for qj in range(NT):
                for hp in range(2):
                    kmax = 4 * qj + 4
                    acc = [psum.tile([65, 512], F32, tag=f"pv{par}", name=f"pv{par}")
                           for par in range(2)]
                    for kt in range(kmax):
                        off = 128 * (kt - 4 * qj) if kt >= 4 * qj else 0
                        sc2 = psum.tile([128, 1024], F32, tag="sc", name="sc2")
                        for par in range(2):
                            nc.tensor.matmul(
                                sc2[:, par * 512 + off:par * 512 + 512],
                                kT[hp][64 * par:64 * par + 64, kt * 128:(kt + 1) * 128],
                                qT[hp][64 * par:64 * par + 64,
                                       qj * 512 + off:(qj + 1) * 512],
                                start=True, stop=True,
                            )
                        pt2 = ptp.tile([128, 2, 512], BF16, tag="pt", name="pt2")
                        nc.scalar.activation(
                            out=pt2[:, :, off:512],
                            in_=sc2[:].rearrange("p (two n) -> p two n", two=2)[:, :, off:512],
                            func=EXP, scale=0.125)
                        if kt >= 4 * qj:
                            for par in range(2):
                                nc.gpsimd.tensor_mul(
                                    pt2[:, par, off:off + 128],
                                    pt2[:, par, off:off + 128],
                                    keep_sb[:, kt * 128:(kt + 1) * 128],
                                )
                        for par in range(2):
                            nc.tensor.matmul(
                                acc[par][:, off:512],
                                vsb[kt][:, 2 * hp + par, :],
                                pt2[:, par, off:512],
                                start=(kt == 0), stop=(kt == kmax - 1),
                            )
                    for par in range(2):
                        h = 2 * hp + par
                        srow = srp.tile([65, 512], F32R, tag="srow", name="srow")
                        nc.vector.tensor_copy(out=srow[64:65, :],
                                              in_=acc[par][64:65, :])
                        rbc = psum.tile([64, 512], F32, tag="sc", name="rbc")
                        nc.tensor.matmul(
                            rbc, ones_r[64:65, :], srow[64:65, :],
                            start=True, stop=True)
                        rinv = rip.tile([64, 512], F32, tag="rinv", name="rinv")
                        nc.vector.reciprocal_approx_fast(out=rinv, in_=rbc)
                        nc.vector.tensor_mul(
                            aTn[h][:, qj * 512:(qj + 1) * 512],
                            acc[par][0:64, :],
                            rinv,
                        )
                if qj > 0:
                    outproj(qj - 1)
            outproj(NT - 1)

    _split_excess_waits(nc)
    nc.compile()
    return nc


def _split_excess_waits(nc):
    """Walrus caps most instructions at 1 sync wait. Peel excess waits off
    matmuls (and anything else over the cap) onto PE-engine wait-nops
    inserted immediately before the instruction."""
    for bb in nc.main_func.blocks:
        new_insts = []
        for inst in bb.instructions:
            si = inst.sync_info
            if (si is not None and si.on_wait and len(si.on_wait) > 1
                    and isinstance(inst, mybir.InstMatmult)):
                excess = list(si.on_wait[:-1])
                keep = [si.on_wait[-1]]
                for w in excess:
                    nop = mybir.InstNoOp(
                        name=nc.get_next_instruction_name(), ins=[], outs=[],
                        bass_nofuse=True)
                    nop.engine = inst.engine
                    nop.sync_info = mybir.SyncInfo(on_wait=[w], on_update=[])
                    nc.register_instruction(nop)
                    new_insts.append(nop)
                si.on_wait = keep
            new_insts.append(inst)
        bb.instructions[:] = new_insts


def _host_prep(x, Wq, Wkv, Wout, mask):
    bf16 = ml_dtypes.bfloat16
    x = np.asarray(x, dtype=np.float32)
    Wq = np.asarray(Wq, dtype=np.float32)
    Wkv = np.asarray(Wkv, dtype=np.float32)
    Wout = np.asarray(Wout, dtype=np.float32)
    mask = np.asarray(mask)

    xT = [np.ascontiguousarray(x[b].T).astype(bf16) for b in range(B)]
    keep = np.empty((128, T), dtype=np.float32)
    for i in range(T // 128):
        blk = mask[128 * i:128 * (i + 1), 128 * i:128 * (i + 1)]
        keep[:, 128 * i:128 * (i + 1)] = (~blk).T.astype(np.float32)
    keep = keep.astype(bf16)

    in_maps = []
    for core in range(NCORES):
        b, g = core // G, core % G
        sl = slice(DG * g, DG * (g + 1))
        in_maps.append({
            "xt": xT[b],
            "wq": np.ascontiguousarray(Wq[sl, :].T).astype(bf16),
            "wk": np.ascontiguousarray(Wkv[sl, :].T).astype(bf16),
            "wv": np.ascontiguousarray(Wkv[C + DG * g:C + DG * (g + 1), :].T).astype(bf16),
            "wo": np.ascontiguousarray(Wout[:, sl].T).astype(bf16),
            "keep": keep,
        })
    return in_maps


def _install_ntff_hook():
    import types
    import antenv
    if getattr(antenv, "axon_hooks", None) is not None:
        return
    ah = types.ModuleType("antenv.axon_hooks")
    ah._hook = None
    ah.set_axon_ntff_profile_hook = lambda h: setattr(ah, "_hook", h)
    ah.get_axon_ntff_profile_hook = lambda: ah._hook
    sys.modules["antenv.axon_hooks"] = ah
    antenv.axon_hooks = ah
    if "/root/.axon_site" not in sys.path:
        sys.path.insert(0, "/root/.axon_site")
    from trn_agent_boot.trn_boot import _ntff_profile_via_ctypes
    ah.set_axon_ntff_profile_hook(_ntff_profile_via_ctypes("/opt/axon/libaxon_pjrt.so"))


def _run(inputs, trace=False):
    global _cached_nc
    from concourse.bass_utils import run_bass_kernel_spmd
    if trace:
        _install_ntff_hook()
    if _cached_nc is None:
        _cached_nc = build_nc()
    in_maps = _host_prep(**inputs)
    res = run_bass_kernel_spmd(_cached_nc, in_maps, list(range(NCORES)), trace=trace)
    parts = [np.asarray(res.results[c]["y"], dtype=np.float32) for c in range(NCORES)]
    out = np.stack([
        parts[0] + parts[1] + parts[2] + parts[3],
        parts[4] + parts[5] + parts[6] + parts[7],
    ]).astype(np.float32)
    return out, res


def kernel(x, Wq, Wkv, Wout, mask):
    out, _ = _run(dict(x=x, Wq=Wq, Wkv=Wkv, Wout=Wout, mask=mask))
    return out


# revision 5
# speedup vs baseline: 1.7653x; 1.1520x over previous
"""Trainium2 Bass kernel for causal MHA (B=2, T=2048, D=1024, H=16, KH=64).

Sharding: 8 cores = 2 (batch) x 4 (head groups of 4 heads).
Each core computes q/k/v projections for its 4 heads, causal attention,
and a partial output projection against its 256-row slice of Wout.
Host sums the 4 partials per batch (the all-reduce step, done at unshard).

v2: bf16 matmul pipeline (fp32r streams trip the power throttler and pin
the PE at 1.2 GHz), fast approximate reciprocal on the broadcast
denominator, pair-batched EXP activations, and output projection
interleaved into the attention loop.
"""
import sys

sys.path.insert(0, "/opt/trn_rl_repo")

from contextlib import ExitStack

import numpy as np
import ml_dtypes

import concourse.bacc as bacc
import concourse.mybir as mybir
import concourse.tile as tile

B, T, C = 2, 2048, 1024
H, KH = 16, 64
G = 4                 # head groups
HPG = H // G          # heads per group = 4
DG = HPG * KH         # 256 per-core head dims
NCORES = 8

F32 = mybir.dt.float32
F32R = mybir.dt.float32r
BF16 = mybir.dt.bfloat16
EXP = mybir.ActivationFunctionType.Exp
COPY = mybir.ActivationFunctionType.Copy

_cached_nc = None


def build_nc():
    nc = bacc.Bacc()
    xt = nc.dram_tensor("xt", [C, T], BF16, kind="ExternalInput")        # x[b].T
    wq = nc.dram_tensor("wq", [C, DG], BF16, kind="ExternalInput")       # Wq slice .T
    wk = nc.dram_tensor("wk", [C, DG], BF16, kind="ExternalInput")
    wv = nc.dram_tensor("wv", [C, DG], BF16, kind="ExternalInput")
    wo = nc.dram_tensor("wo", [DG, C], BF16, kind="ExternalInput")       # Wout[:, slice].T
    keep = nc.dram_tensor("keep", [128, T], BF16, kind="ExternalInput")  # diag keep (k, q)
    y = nc.dram_tensor("y", [T, C], BF16, kind="ExternalOutput")         # partial output

    NT = T // 512     # 4 moving t tiles
    NK = C // 128     # 8 contraction chunks
    NTT = T // 128    # 16 t tiles of 128

    with ExitStack() as ctx:
        ctx.enter_context(nc.allow_low_precision(reason="bf16 matmul pipeline"))
        tc = ctx.enter_context(tile.TileContext(nc))
        persist = ctx.enter_context(tc.tile_pool(name="persist", bufs=1))
        psum = ctx.enter_context(tc.tile_pool(name="psum", bufs=2, space="PSUM"))

        # ---- persistent tiles ----
        qT = [persist.tile([128, T], BF16, tag=f"qT{i}", name=f"qT{i}") for i in range(2)]
        kT = [persist.tile([128, T], BF16, tag=f"kT{i}", name=f"kT{i}") for i in range(2)]
        vsb = [persist.tile([128, HPG, KH + 1], BF16, tag=f"v{i}", name=f"v{i}")
               for i in range(NTT)]
        aT2 = [persist.tile([128, T], BF16, tag=f"aT2{g}", name=f"aT2{g}")
               for g in range(2)]
        wo2 = [persist.tile([128, C], BF16, tag=f"wo2{g}", name=f"wo2{g}")
               for g in range(2)]
        keep_sb = persist.tile([128, T], BF16, tag="keep")
        ones_r = persist.tile([65, 64], F32R, tag="ones_r")
        ones_f32 = persist.tile([65, 64], F32, tag="ones_f32")

        nc.vector.memset(ones_f32, 1.0)
        nc.vector.tensor_copy(out=ones_r, in_=ones_f32)

        # ================= Phase 1: projections =================
        with tc.tile_pool(name="ph1", bufs=1) as ph1:
            xT = ph1.tile([128, NK, T], BF16, tag="xT", name="xT")
            wq_sb = ph1.tile([128, NK, DG], BF16, tag="wq_sb", name="wq_sb")
            wk_sb = ph1.tile([128, NK, DG], BF16, tag="wk_sb", name="wk_sb")
            wv_sb = ph1.tile([128, NK, DG], BF16, tag="wv_sb", name="wv_sb")
            # DMA issue order = first-use order: wq, x(t-block 0), wk, wv,
            # remaining x blocks, then mask + Wout (needed only in phase 2/3).
            xt_r = xt.rearrange("(k p) t -> p k t", p=128)
            nc.sync.dma_start(out=wq_sb, in_=wq.rearrange("(k p) d -> p k d", p=128))
            nc.sync.dma_start(out=xT[:, :, 0:512], in_=xt_r[:, :, 0:512])
            nc.sync.dma_start(out=wk_sb, in_=wk.rearrange("(k p) d -> p k d", p=128))
            nc.sync.dma_start(out=wv_sb, in_=wv.rearrange("(k p) d -> p k d", p=128))
            for n in range(1, NT):
                nc.sync.dma_start(out=xT[:, :, n * 512:(n + 1) * 512],
                                  in_=xt_r[:, :, n * 512:(n + 1) * 512])
            nc.sync.dma_start(out=keep_sb, in_=keep[:, :])
            for g in range(2):
                nc.sync.dma_start(out=wo2[g], in_=wo[g * 128:(g + 1) * 128, :])

            # qT/kT: (dk 128-block, t) = sum_c w[c, dk].T . xT[c, t]
            for n in range(NT):           # moving t tile of 512
                for dst, w_sb in ((qT, wq_sb), (kT, wk_sb)):
                    for m in range(2):    # head pair -> partition block
                        ps = psum.tile([128, 512], F32, tag="sc", name="ps")
                        for k in range(NK):
                            nc.tensor.matmul(
                                ps,
                                w_sb[:, k, m * 128:(m + 1) * 128],
                                xT[:, k, n * 512:(n + 1) * 512],
                                start=(k == 0), stop=(k == NK - 1),
                            )
                        nc.scalar.activation(
                            out=dst[m][:, n * 512:(n + 1) * 512], in_=ps, func=COPY)
                # V: (t 128, dv 256) = sum_c xT[c, t].T . wv[c, dv]
                for i in range(4):
                    tt = 4 * n + i
                    ps = psum.tile([128, DG], F32, tag="sc", name="ps")
                    for k in range(NK):
                        nc.tensor.matmul(
                            ps,
                            xT[:, k, tt * 128:(tt + 1) * 128],
                            wv_sb[:, k, :],
                            start=(k == 0), stop=(k == NK - 1),
                        )
                    nc.vector.tensor_copy(
                        out=vsb[tt][:, :, 0:KH],
                        in_=ps[:].rearrange("p (h d) -> p h d", h=HPG),
                    )
                    nc.gpsimd.memset(vsb[tt][:, :, KH:KH + 1], 1.0)

        # ============ Phase 2+3: attention, outproj interleaved ============
        def outproj(qj):
            for i in range(4):
                tt = 4 * qj + i
                yt = ytp.tile([128, C], BF16, tag="yt", name="yt")
                for no in range(2):
                    yp = psum.tile([128, 512], F32, tag="sc", name="yp")
                    for g in range(2):
                        nc.tensor.matmul(
                            yp,
                            aT2[g][:, tt * 128:(tt + 1) * 128],
                            wo2[g][:, no * 512:(no + 1) * 512],
                            start=(g == 0), stop=(g == 1),
                        )
                    nc.vector.tensor_copy(out=yt[:, no * 512:(no + 1) * 512], in_=yp)
                nc.sync.dma_start(out=y[tt * 128:(tt + 1) * 128, :], in_=yt)

        with tc.tile_pool(name="pts", bufs=4) as ptp, \
             tc.tile_pool(name="srowp", bufs=4) as srp, \
             tc.tile_pool(name="rinvp", bufs=2) as rip, \
             tc.tile_pool(name="ytp", bufs=2) as ytp:
            for qj in range(NT):
                for hp in range(2):
                    kmax = 4 * qj + 4
                    acc = [psum.tile([65, 512], F32, tag=f"pv{par}", name=f"pv{par}")
                           for par in range(2)]
                    for kt in range(kmax):
                        off = 128 * (kt - 4 * qj) if kt >= 4 * qj else 0
                        sc2 = psum.tile([128, 1024], F32, tag="sc", name="sc2")
                        for par in range(2):
                            nc.tensor.matmul(
                                sc2[:, par * 512 + off:par * 512 + 512],
                                kT[hp][64 * par:64 * par + 64, kt * 128:(kt + 1) * 128],
                                qT[hp][64 * par:64 * par + 64,
                                       qj * 512 + off:(qj + 1) * 512],
                                start=True, stop=True,
                            )
                        pt2 = ptp.tile([128, 2, 512], BF16, tag="pt", name="pt2")
                        nc.scalar.activation(
                            out=pt2[:, :, off:512],
                            in_=sc2[:].rearrange("p (two n) -> p two n", two=2)[:, :, off:512],
                            func=EXP, scale=0.125)
                        if kt >= 4 * qj:
                            for par in range(2):
                                nc.gpsimd.tensor_mul(
                                    pt2[:, par, off:off + 128],
                                    pt2[:, par, off:off + 128],
                                    keep_sb[:, kt * 128:(kt + 1) * 128],
                                )
                        for par in range(2):
                            nc.tensor.matmul(
                                acc[par][:, off:512],
                                vsb[kt][:, 2 * hp + par, :],
                                pt2[:, par, off:512],
                                start=(kt == 0), stop=(kt == kmax - 1),
                            )
                    for par in range(2):
                        srow = srp.tile([65, 512], F32R, tag="srow", name="srow")
                        nc.vector.tensor_copy(out=srow[64:65, :],
                                              in_=acc[par][64:65, :])
                        rbc = psum.tile([64, 512], F32, tag="sc", name="rbc")
                        nc.tensor.matmul(
                            rbc, ones_r[64:65, :], srow[64:65, :],
                            start=True, stop=True)
                        rinv = rip.tile([64, 512], F32, tag="rinv", name="rinv")
                        nc.vector.reciprocal_approx_fast(out=rinv, in_=rbc)
                        nc.vector.tensor_mul(
                            aT2[hp][64 * par:64 * par + 64,
                                    qj * 512:(qj + 1) * 512],
                            acc[par][0:64, :],
                            rinv,
                        )
                if qj > 0:
                    outproj(qj - 1)
            outproj(NT - 1)

    _split_excess_waits(nc)
    nc.compile()
    return nc


def _split_excess_waits(nc):
    """Walrus caps most instructions at 1 sync wait. Peel excess waits off
    matmuls (and anything else over the cap) onto PE-engine wait-nops
    inserted immediately before the instruction."""
    for bb in nc.main_func.blocks:
        new_insts = []
        for inst in bb.instructions:
            si = inst.sync_info
            if (si is not None and si.on_wait and len(si.on_wait) > 1
                    and isinstance(inst, mybir.InstMatmult)):
                excess = list(si.on_wait[:-1])
                keep = [si.on_wait[-1]]
                for w in excess:
                    nop = mybir.InstNoOp(
                        name=nc.get_next_instruction_name(), ins=[], outs=[],
                        bass_nofuse=True)
                    nop.engine = inst.engine
                    nop.sync_info = mybir.SyncInfo(on_wait=[w], on_update=[])
                    nc.register_instruction(nop)
                    new_insts.append(nop)
                si.on_wait = keep
            new_insts.append(inst)
        bb.instructions[:] = new_insts


def _host_prep(x, Wq, Wkv, Wout, mask):
    bf16 = ml_dtypes.bfloat16
    x = np.asarray(x, dtype=np.float32)
    Wq = np.asarray(Wq, dtype=np.float32)
    Wkv = np.asarray(Wkv, dtype=np.float32)
    Wout = np.asarray(Wout, dtype=np.float32)
    mask = np.asarray(mask)

    xT = [np.ascontiguousarray(x[b].T).astype(bf16) for b in range(B)]
    keep = np.empty((128, T), dtype=np.float32)
    for i in range(T // 128):
        blk = mask[128 * i:128 * (i + 1), 128 * i:128 * (i + 1)]
        keep[:, 128 * i:128 * (i + 1)] = (~blk).T.astype(np.float32)
    keep = keep.astype(bf16)

    in_maps = []
    for core in range(NCORES):
        b, g = core // G, core % G
        sl = slice(DG * g, DG * (g + 1))
        in_maps.append({
            "xt": xT[b],
            "wq": np.ascontiguousarray(Wq[sl, :].T).astype(bf16),
            "wk": np.ascontiguousarray(Wkv[sl, :].T).astype(bf16),
            "wv": np.ascontiguousarray(Wkv[C + DG * g:C + DG * (g + 1), :].T).astype(bf16),
            "wo": np.ascontiguousarray(Wout[:, sl].T).astype(bf16),
            "keep": keep,
        })
    return in_maps


def _install_ntff_hook():
    import types
    import antenv
    if getattr(antenv, "axon_hooks", None) is not None:
        return
    ah = types.ModuleType("antenv.axon_hooks")
    ah._hook = None
    ah.set_axon_ntff_profile_hook = lambda h: setattr(ah, "_hook", h)
    ah.get_axon_ntff_profile_hook = lambda: ah._hook
    sys.modules["antenv.axon_hooks"] = ah
    antenv.axon_hooks = ah
    if "/root/.axon_site" not in sys.path:
        sys.path.insert(0, "/root/.axon_site")
    from trn_agent_boot.trn_boot import _ntff_profile_via_ctypes
    ah.set_axon_ntff_profile_hook(_ntff_profile_via_ctypes("/opt/axon/libaxon_pjrt.so"))


def _run(inputs, trace=False):
    global _cached_nc
    from concourse.bass_utils import run_bass_kernel_spmd
    if trace:
        _install_ntff_hook()
    if _cached_nc is None:
        _cached_nc = build_nc()
    in_maps = _host_prep(**inputs)
    res = run_bass_kernel_spmd(_cached_nc, in_maps, list(range(NCORES)), trace=trace)
    parts = [np.asarray(res.results[c]["y"], dtype=np.float32) for c in range(NCORES)]
    out = np.stack([
        parts[0] + parts[1] + parts[2] + parts[3],
        parts[4] + parts[5] + parts[6] + parts[7],
    ]).astype(np.float32)
    return out, res


def kernel(x, Wq, Wkv, Wout, mask):
    out, _ = _run(dict(x=x, Wq=Wq, Wkv=Wkv, Wout=Wout, mask=mask))
    return out


# revision 8
# speedup vs baseline: 1.9424x; 1.1003x over previous
"""Trainium2 Bass kernel for causal MHA (B=2, T=2048, D=1024, H=16, KH=64).

Sharding: 8 cores = 2 (batch) x 4 (head groups of 4 heads).
Each core computes q/k/v projections for its 4 heads, causal attention,
and a partial output projection against its 256-row slice of Wout.
Host sums the 4 partials per batch (the all-reduce step, done at unshard).

v2: bf16 matmul pipeline (fp32r streams trip the power throttler and pin
the PE at 1.2 GHz), fast approximate reciprocal on the broadcast
denominator, pair-batched EXP activations, and output projection
interleaved into the attention loop.
"""
import sys

sys.path.insert(0, "/opt/trn_rl_repo")

from contextlib import ExitStack

import numpy as np
import ml_dtypes

import concourse.bacc as bacc
import concourse.mybir as mybir
import concourse.tile as tile

B, T, C = 2, 2048, 1024
H, KH = 16, 64
G = 4                 # head groups
HPG = H // G          # heads per group = 4
DG = HPG * KH         # 256 per-core head dims
NCORES = 8

F32 = mybir.dt.float32
F32R = mybir.dt.float32r
BF16 = mybir.dt.bfloat16
EXP = mybir.ActivationFunctionType.Exp
COPY = mybir.ActivationFunctionType.Copy

_cached_nc = None


def build_nc():
    nc = bacc.Bacc()
    xt = nc.dram_tensor("xt", [C, T], BF16, kind="ExternalInput")        # x[b].T
    wq = nc.dram_tensor("wq", [C, DG], BF16, kind="ExternalInput")       # Wq slice .T
    wk = nc.dram_tensor("wk", [C, DG], BF16, kind="ExternalInput")
    wv = nc.dram_tensor("wv", [C, DG], BF16, kind="ExternalInput")
    wo = nc.dram_tensor("wo", [DG, C], BF16, kind="ExternalInput")       # Wout[:, slice].T
    keep = nc.dram_tensor("keep", [128, T], BF16, kind="ExternalInput")  # diag keep (k, q)
    y = nc.dram_tensor("y", [T, C], BF16, kind="ExternalOutput")         # partial output

    NT = T // 512     # 4 moving t tiles
    NK = C // 128     # 8 contraction chunks
    NTT = T // 128    # 16 t tiles of 128

    with ExitStack() as ctx:
        ctx.enter_context(nc.allow_low_precision(reason="bf16 matmul pipeline"))
        tc = ctx.enter_context(tile.TileContext(nc))
        persist = ctx.enter_context(tc.tile_pool(name="persist", bufs=1))
        psum = ctx.enter_context(tc.tile_pool(name="psum", bufs=2, space="PSUM"))

        # ---- persistent tiles ----
        qT = [persist.tile([128, T], BF16, tag=f"qT{i}", name=f"qT{i}") for i in range(2)]
        kT = [persist.tile([128, T], BF16, tag=f"kT{i}", name=f"kT{i}") for i in range(2)]
        vsb = [persist.tile([128, HPG, KH + 1], BF16, tag=f"v{i}", name=f"v{i}")
               for i in range(NTT)]
        aT2 = [persist.tile([128, T], BF16, tag=f"aT2{g}", name=f"aT2{g}")
               for g in range(2)]
        wo2 = [persist.tile([128, C], BF16, tag=f"wo2{g}", name=f"wo2{g}")
               for g in range(2)]
        keep_sb = persist.tile([128, T], BF16, tag="keep")
        ones_r = persist.tile([65, 64], F32R, tag="ones_r")
        ones_f32 = persist.tile([65, 64], F32, tag="ones_f32")

        nc.vector.memset(ones_f32, 1.0)
        nc.vector.tensor_copy(out=ones_r, in_=ones_f32)

        # ================= Phase 1: projections =================
        with tc.tile_pool(name="ph1", bufs=1) as ph1:
            xT = ph1.tile([128, NK, T], BF16, tag="xT", name="xT")
            wq_sb = ph1.tile([128, NK, DG], BF16, tag="wq_sb", name="wq_sb")
            wk_sb = ph1.tile([128, NK, DG], BF16, tag="wk_sb", name="wk_sb")
            wv_sb = ph1.tile([128, NK, DG], BF16, tag="wv_sb", name="wv_sb")
            # DMA issue order = first-use order. Split the head of the x
            # load so the first matmul group is fed as early as possible.
            xt_r = xt.rearrange("(k p) t -> p k t", p=128)
            wq_r = wq.rearrange("(k p) d -> p k d", p=128)
            nc.sync.dma_start(out=wq_sb[:, :, 0:128], in_=wq_r[:, :, 0:128])
            nc.sync.dma_start(out=xT[:, 0:4, 0:512], in_=xt_r[:, 0:4, 0:512])
            nc.sync.dma_start(out=xT[:, 4:8, 0:512], in_=xt_r[:, 4:8, 0:512])
            nc.sync.dma_start(out=wq_sb[:, :, 128:256], in_=wq_r[:, :, 128:256])
            nc.sync.dma_start(out=wk_sb, in_=wk.rearrange("(k p) d -> p k d", p=128))
            nc.sync.dma_start(out=wv_sb, in_=wv.rearrange("(k p) d -> p k d", p=128))
            for n in range(1, NT):
                nc.sync.dma_start(out=xT[:, :, n * 512:(n + 1) * 512],
                                  in_=xt_r[:, :, n * 512:(n + 1) * 512])
            nc.sync.dma_start(out=keep_sb, in_=keep[:, :])
            for g in range(2):
                nc.sync.dma_start(out=wo2[g], in_=wo[g * 128:(g + 1) * 128, :])

            # qT/kT: (dk 128-block, t) = sum_c w[c, dk].T . xT[c, t]
            for n in range(NT):           # moving t tile of 512
                for dst, w_sb in ((qT, wq_sb), (kT, wk_sb)):
                    for m in range(2):    # head pair -> partition block
                        ps = psum.tile([128, 512], F32, tag="sc", name="ps")
                        for k in range(NK):
                            nc.tensor.matmul(
                                ps,
                                w_sb[:, k, m * 128:(m + 1) * 128],
                                xT[:, k, n * 512:(n + 1) * 512],
                                start=(k == 0), stop=(k == NK - 1),
                            )
                        nc.scalar.activation(
                            out=dst[m][:, n * 512:(n + 1) * 512], in_=ps, func=COPY)
                # V: (t 128, dv 256) = sum_c xT[c, t].T . wv[c, dv]
                for i in range(4):
                    tt = 4 * n + i
                    ps = psum.tile([128, DG], F32, tag="sc", name="ps")
                    for k in range(NK):
                        nc.tensor.matmul(
                            ps,
                            xT[:, k, tt * 128:(tt + 1) * 128],
                            wv_sb[:, k, :],
                            start=(k == 0), stop=(k == NK - 1),
                        )
                    nc.vector.tensor_copy(
                        out=vsb[tt][:, :, 0:KH],
                        in_=ps[:].rearrange("p (h d) -> p h d", h=HPG),
                    )
                    nc.gpsimd.memset(vsb[tt][:, :, KH:KH + 1], 1.0)

        # ============ Phase 2+3: attention, outproj interleaved ============
        def outproj(qj, last=False):
            for i in range(4):
                tt = 4 * qj + i
                yt = ytp.tile([128, C], BF16, tag="yt", name="yt")
                for no in range(2):
                    # yp rides the pv slots: the matching attention acc buf
                    # was released by qj's normalization two iterations ago,
                    # so this never stalls the score (sc) pipeline.
                    yp = psum.tile([128, 512], F32, tag=f"pv{no}", name="yp")
                    for g in range(2):
                        nc.tensor.matmul(
                            yp,
                            aT2[g][:, tt * 128:(tt + 1) * 128],
                            wo2[g][:, no * 512:(no + 1) * 512],
                            start=(g == 0), stop=(g == 1),
                        )
                    if last and no == 1:
                        # drain the kernel tail on two engines in parallel
                        nc.scalar.activation(
                            out=yt[:, no * 512:(no + 1) * 512], in_=yp, func=COPY)
                    else:
                        nc.vector.tensor_copy(
                            out=yt[:, no * 512:(no + 1) * 512], in_=yp)
                    nc.sync.dma_start(
                        out=y[tt * 128:(tt + 1) * 128, no * 512:(no + 1) * 512],
                        in_=yt[:, no * 512:(no + 1) * 512])

        with tc.tile_pool(name="pts", bufs=4) as ptp, \
             tc.tile_pool(name="srowp", bufs=4) as srp, \
             tc.tile_pool(name="rinvp", bufs=2) as rip, \
             tc.tile_pool(name="ytp", bufs=2) as ytp:
            for qj in range(NT):
                for hp in range(2):
                    kmax = 4 * qj + 4
                    acc = [psum.tile([65, 512], F32, tag=f"pv{par}", name=f"pv{par}")
                           for par in range(2)]
                    for kt in range(kmax):
                        off = 128 * (kt - 4 * qj) if kt >= 4 * qj else 0
                        sc2 = psum.tile([128, 1024], F32, tag="sc", name="sc2")
                        for par in range(2):
                            nc.tensor.matmul(
                                sc2[:, par * 512 + off:par * 512 + 512],
                                kT[hp][64 * par:64 * par + 64, kt * 128:(kt + 1) * 128],
                                qT[hp][64 * par:64 * par + 64,
                                       qj * 512 + off:(qj + 1) * 512],
                                start=True, stop=True,
                            )
                        pt2 = ptp.tile([128, 2, 512], BF16, tag="pt", name="pt2")
                        nc.scalar.activation(
                            out=pt2[:, :, off:512],
                            in_=sc2[:].rearrange("p (two n) -> p two n", two=2)[:, :, off:512],
                            func=EXP, scale=0.125)
                        if kt >= 4 * qj:
                            for par in range(2):
                                nc.gpsimd.tensor_mul(
                                    pt2[:, par, off:off + 128],
                                    pt2[:, par, off:off + 128],
                                    keep_sb[:, kt * 128:(kt + 1) * 128],
                                )
                        for par in range(2):
                            nc.tensor.matmul(
                                acc[par][:, off:512],
                                vsb[kt][:, 2 * hp + par, :],
                                pt2[:, par, off:512],
                                start=(kt == 0), stop=(kt == kmax - 1),
                            )
                    for par in range(2):
                        srow = srp.tile([65, 512], F32R, tag="srow", name="srow")
                        nc.vector.tensor_copy(out=srow[64:65, :],
                                              in_=acc[par][64:65, :])
                        rbc = psum.tile([64, 512], F32, tag="sc", name="rbc")
                        nc.tensor.matmul(
                            rbc, ones_r[64:65, :], srow[64:65, :],
                            start=True, stop=True)
                        rinv = rip.tile([64, 512], F32, tag="rinv", name="rinv")
                        nc.vector.reciprocal_approx_fast(out=rinv, in_=rbc)
                        nc.vector.tensor_mul(
                            aT2[hp][64 * par:64 * par + 64,
                                    qj * 512:(qj + 1) * 512],
                            acc[par][0:64, :],
                            rinv,
                        )
                if qj > 0:
                    outproj(qj - 1)
            outproj(NT - 1, last=True)

    _split_excess_waits(nc)
    nc.compile()
    return nc


def _split_excess_waits(nc):
    """Walrus caps most instructions at 1 sync wait. Peel excess waits off
    matmuls (and anything else over the cap) onto PE-engine wait-nops
    inserted immediately before the instruction."""
    for bb in nc.main_func.blocks:
        new_insts = []
        for inst in bb.instructions:
            si = inst.sync_info
            if (si is not None and si.on_wait and len(si.on_wait) > 1
                    and isinstance(inst, mybir.InstMatmult)):
                excess = list(si.on_wait[:-1])
                keep = [si.on_wait[-1]]
                for w in excess:
                    nop = mybir.InstNoOp(
                        name=nc.get_next_instruction_name(), ins=[], outs=[],
                        bass_nofuse=True)
                    nop.engine = inst.engine
                    nop.sync_info = mybir.SyncInfo(on_wait=[w], on_update=[])
                    nc.register_instruction(nop)
                    new_insts.append(nop)
                si.on_wait = keep
            new_insts.append(inst)
        bb.instructions[:] = new_insts


def _host_prep(x, Wq, Wkv, Wout, mask):
    bf16 = ml_dtypes.bfloat16
    x = np.asarray(x, dtype=np.float32)
    Wq = np.asarray(Wq, dtype=np.float32)
    Wkv = np.asarray(Wkv, dtype=np.float32)
    Wout = np.asarray(Wout, dtype=np.float32)
    mask = np.asarray(mask)

    xT = [np.ascontiguousarray(x[b].T).astype(bf16) for b in range(B)]
    keep = np.empty((128, T), dtype=np.float32)
    for i in range(T // 128):
        blk = mask[128 * i:128 * (i + 1), 128 * i:128 * (i + 1)]
        keep[:, 128 * i:128 * (i + 1)] = (~blk).T.astype(np.float32)
    keep = keep.astype(bf16)

    in_maps = []
    for core in range(NCORES):
        b, g = core // G, core % G
        sl = slice(DG * g, DG * (g + 1))
        in_maps.append({
            "xt": xT[b],
            "wq": np.ascontiguousarray(Wq[sl, :].T).astype(bf16),
            "wk": np.ascontiguousarray(Wkv[sl, :].T).astype(bf16),
            "wv": np.ascontiguousarray(Wkv[C + DG * g:C + DG * (g + 1), :].T).astype(bf16),
            "wo": np.ascontiguousarray(Wout[:, sl].T).astype(bf16),
            "keep": keep,
        })
    return in_maps


def _install_ntff_hook():
    import types
    import antenv
    if getattr(antenv, "axon_hooks", None) is not None:
        return
    ah = types.ModuleType("antenv.axon_hooks")
    ah._hook = None
    ah.set_axon_ntff_profile_hook = lambda h: setattr(ah, "_hook", h)
    ah.get_axon_ntff_profile_hook = lambda: ah._hook
    sys.modules["antenv.axon_hooks"] = ah
    antenv.axon_hooks = ah
    if "/root/.axon_site" not in sys.path:
        sys.path.insert(0, "/root/.axon_site")
    from trn_agent_boot.trn_boot import _ntff_profile_via_ctypes
    ah.set_axon_ntff_profile_hook(_ntff_profile_via_ctypes("/opt/axon/libaxon_pjrt.so"))


def _run(inputs, trace=False):
    global _cached_nc
    from concourse.bass_utils import run_bass_kernel_spmd
    if trace:
        _install_ntff_hook()
    if _cached_nc is None:
        _cached_nc = build_nc()
    in_maps = _host_prep(**inputs)
    res = run_bass_kernel_spmd(_cached_nc, in_maps, list(range(NCORES)), trace=trace)
    parts = [np.asarray(res.results[c]["y"], dtype=np.float32) for c in range(NCORES)]
    out = np.stack([
        parts[0] + parts[1] + parts[2] + parts[3],
        parts[4] + parts[5] + parts[6] + parts[7],
    ]).astype(np.float32)
    return out, res


def kernel(x, Wq, Wkv, Wout, mask):
    out, _ = _run(dict(x=x, Wq=Wq, Wkv=Wkv, Wout=Wout, mask=mask))
    return out


# revision 9
# speedup vs baseline: 1.9530x; 1.0054x over previous
"""Trainium2 Bass kernel for causal MHA (B=2, T=2048, D=1024, H=16, KH=64).

Sharding: 8 cores = 2 (batch) x 4 (head groups of 4 heads).
Each core computes q/k/v projections for its 4 heads, causal attention,
and a partial output projection against its 256-row slice of Wout.
Host sums the 4 partials per batch (the all-reduce step, done at unshard).

v5: bf16 matmul pipeline end to end; q/k/v projection chains interleaved
into the attention stream as PE filler (the attention stretch is paced by
the scalar-engine EXP chain, so a separate projection phase both serializes
the walls and lets the PE clock-gate re-throttle); fast approximate
reciprocal on the matmul-broadcast denominator; pair-batched EXP; output
projection of block qj-1 riding its own PSUM slots inside block qj.
"""
import sys

sys.path.insert(0, "/opt/trn_rl_repo")

from contextlib import ExitStack

import numpy as np
import ml_dtypes

import concourse.bacc as bacc
import concourse.mybir as mybir
import concourse.tile as tile

B, T, C = 2, 2048, 1024
H, KH = 16, 64
G = 4                 # head groups
HPG = H // G          # heads per group = 4
DG = HPG * KH         # 256 per-core head dims
NCORES = 8

F32 = mybir.dt.float32
F32R = mybir.dt.float32r
BF16 = mybir.dt.bfloat16
EXP = mybir.ActivationFunctionType.Exp
COPY = mybir.ActivationFunctionType.Copy

_cached_nc = None


def build_nc():
    nc = bacc.Bacc()
    xt = nc.dram_tensor("xt", [C, T], BF16, kind="ExternalInput")        # x[b].T
    wq = nc.dram_tensor("wq", [C, DG], BF16, kind="ExternalInput")       # Wq slice .T
    wk = nc.dram_tensor("wk", [C, DG], BF16, kind="ExternalInput")
    wv = nc.dram_tensor("wv", [C, DG], BF16, kind="ExternalInput")
    wo = nc.dram_tensor("wo", [DG, C], BF16, kind="ExternalInput")       # Wout[:, slice].T
    keep = nc.dram_tensor("keep", [128, T], BF16, kind="ExternalInput")  # diag keep (k, q)
    y = nc.dram_tensor("y", [T, C], BF16, kind="ExternalOutput")         # partial output

    NT = T // 512     # 4 t blocks
    NK = C // 128     # 8 contraction chunks
    NTT = T // 128    # 16 t tiles of 128

    with ExitStack() as ctx:
        ctx.enter_context(nc.allow_low_precision(reason="bf16 matmul pipeline"))
        tc = ctx.enter_context(tile.TileContext(nc))
        persist = ctx.enter_context(tc.tile_pool(name="persist", bufs=1))
        psum = ctx.enter_context(tc.tile_pool(name="psum", bufs=2, space="PSUM"))

        # ---- persistent tiles ----
        qT = [persist.tile([128, T], BF16, tag=f"qT{i}", name=f"qT{i}") for i in range(2)]
        kT = [persist.tile([128, T], BF16, tag=f"kT{i}", name=f"kT{i}") for i in range(2)]
        vsb = [persist.tile([128, HPG, KH + 1], BF16, tag=f"v{i}", name=f"v{i}")
               for i in range(NTT)]
        aT2 = [persist.tile([128, T], BF16, tag=f"aT2{g}", name=f"aT2{g}")
               for g in range(2)]
        wo2 = [persist.tile([128, C], BF16, tag=f"wo2{g}", name=f"wo2{g}")
               for g in range(2)]
        keep_sb = persist.tile([128, T], BF16, tag="keep")
        ones_r = persist.tile([65, 64], F32R, tag="ones_r")
        ones_f32 = persist.tile([65, 64], F32, tag="ones_f32")
        xT = persist.tile([128, NK, T], BF16, tag="xT", name="xT")
        wq_sb = persist.tile([128, NK, DG], BF16, tag="wq_sb", name="wq_sb")
        wk_sb = persist.tile([128, NK, DG], BF16, tag="wk_sb", name="wk_sb")
        wv_sb = persist.tile([128, NK, DG], BF16, tag="wv_sb", name="wv_sb")

        # DMA issue order = first-use order. Split the head of the x load so
        # the first matmul group is fed as early as possible.
        xt_r = xt.rearrange("(k p) t -> p k t", p=128)
        wq_r = wq.rearrange("(k p) d -> p k d", p=128)
        nc.sync.dma_start(out=wq_sb[:, :, 0:128], in_=wq_r[:, :, 0:128])
        nc.sync.dma_start(out=xT[:, 0:4, 0:512], in_=xt_r[:, 0:4, 0:512])
        nc.sync.dma_start(out=xT[:, 4:8, 0:512], in_=xt_r[:, 4:8, 0:512])
        nc.sync.dma_start(out=wq_sb[:, :, 128:256], in_=wq_r[:, :, 128:256])
        nc.sync.dma_start(out=wk_sb, in_=wk.rearrange("(k p) d -> p k d", p=128))
        nc.sync.dma_start(out=keep_sb, in_=keep[:, :])
        nc.sync.dma_start(out=wv_sb, in_=wv.rearrange("(k p) d -> p k d", p=128))
        for n in range(1, NT):
            nc.sync.dma_start(out=xT[:, :, n * 512:(n + 1) * 512],
                              in_=xt_r[:, :, n * 512:(n + 1) * 512])
        for g in range(2):
            nc.sync.dma_start(out=wo2[g], in_=wo[g * 128:(g + 1) * 128, :])

        nc.vector.memset(ones_f32, 1.0)
        nc.vector.tensor_copy(out=ones_r, in_=ones_f32)
        for tt in range(NTT):
            nc.gpsimd.memset(vsb[tt][:, :, KH:KH + 1], 1.0)

        # ---- projection chain units (PE filler inside the qj loop) ----
        def qk_unit(dst, w_sb, m, n):
            ps = psum.tile([128, 512], F32, tag="pj", name="ps")
            for k in range(NK):
                nc.tensor.matmul(
                    ps,
                    w_sb[:, k, m * 128:(m + 1) * 128],
                    xT[:, k, n * 512:(n + 1) * 512],
                    start=(k == 0), stop=(k == NK - 1),
                )
            nc.vector.tensor_copy(out=dst[m][:, n * 512:(n + 1) * 512], in_=ps)

        def v_unit(tt):
            ps = psum.tile([128, DG], F32, tag="pj", name="ps")
            for k in range(NK):
                nc.tensor.matmul(
                    ps,
                    xT[:, k, tt * 128:(tt + 1) * 128],
                    wv_sb[:, k, :],
                    start=(k == 0), stop=(k == NK - 1),
                )
            nc.vector.tensor_copy(
                out=vsb[tt][:, :, 0:KH],
                in_=ps[:].rearrange("p (h d) -> p h d", h=HPG),
            )

        def proj_units(n):
            units = []
            for dst, w_sb in ((qT, wq_sb), (kT, wk_sb)):
                for m in range(2):
                    units.append(lambda dst=dst, w_sb=w_sb, m=m: qk_unit(dst, w_sb, m, n))
            for i in range(4):
                units.append(lambda tt=4 * n + i: v_unit(tt))
            return units

        def attn_unit(qj, hp):
            kmax = 4 * qj + 4
            acc = [psum.tile([65, 512], F32, tag=f"pv{par}", bufs=1,
                             name=f"pv{par}")
                   for par in range(2)]
            for kt in range(kmax):
                off = 128 * (kt - 4 * qj) if kt >= 4 * qj else 0
                sc2 = psum.tile([128, 1024], F32, tag="sc", name="sc2")
                for par in range(2):
                    nc.tensor.matmul(
                        sc2[:, par * 512 + off:par * 512 + 512],
                        kT[hp][64 * par:64 * par + 64, kt * 128:(kt + 1) * 128],
                        qT[hp][64 * par:64 * par + 64,
                               qj * 512 + off:(qj + 1) * 512],
                        start=True, stop=True,
                    )
                pt2 = ptp.tile([128, 2, 512], BF16, tag="pt", name="pt2")
                nc.scalar.activation(
                    out=pt2[:, :, off:512],
                    in_=sc2[:].rearrange("p (two n) -> p two n", two=2)[:, :, off:512],
                    func=EXP, scale=0.125)
                if kt >= 4 * qj:
                    for par in range(2):
                        nc.gpsimd.tensor_mul(
                            pt2[:, par, off:off + 128],
                            pt2[:, par, off:off + 128],
                            keep_sb[:, kt * 128:(kt + 1) * 128],
                        )
                for par in range(2):
                    nc.tensor.matmul(
                        acc[par][:, off:512],
                        vsb[kt][:, 2 * hp + par, :],
                        pt2[:, par, off:512],
                        start=(kt == 0), stop=(kt == kmax - 1),
                    )
            for par in range(2):
                srow = srp.tile([65, 512], F32R, tag="srow", name="srow")
                nc.vector.tensor_copy(out=srow[64:65, :], in_=acc[par][64:65, :])
                rbc = psum.tile([64, 512], F32, tag="pj", name="rbc")
                nc.tensor.matmul(
                    rbc, ones_r[64:65, :], srow[64:65, :], start=True, stop=True)
                rinv = rip.tile([64, 512], F32, tag="rinv", name="rinv")
                nc.vector.reciprocal_approx_fast(out=rinv, in_=rbc)
                nc.vector.tensor_mul(
                    aT2[hp][64 * par:64 * par + 64, qj * 512:(qj + 1) * 512],
                    acc[par][0:64, :],
                    rinv,
                )

        def outproj(qj, last=False):
            for i in range(4):
                tt = 4 * qj + i
                yt = ytp.tile([128, C], BF16, tag="yt", name="yt")
                for no in range(2):
                    yp = psum.tile([128, 512], F32, tag="pj", name="yp")
                    for g in range(2):
                        nc.tensor.matmul(
                            yp,
                            aT2[g][:, tt * 128:(tt + 1) * 128],
                            wo2[g][:, no * 512:(no + 1) * 512],
                            start=(g == 0), stop=(g == 1),
                        )
                    if last and no == 1:
                        # drain the kernel tail on two engines in parallel
                        nc.scalar.activation(
                            out=yt[:, no * 512:(no + 1) * 512], in_=yp, func=COPY)
                    else:
                        nc.vector.tensor_copy(
                            out=yt[:, no * 512:(no + 1) * 512], in_=yp)
                    nc.sync.dma_start(
                        out=y[tt * 128:(tt + 1) * 128, no * 512:(no + 1) * 512],
                        in_=yt[:, no * 512:(no + 1) * 512])

        with tc.tile_pool(name="pts", bufs=6) as ptp, \
             tc.tile_pool(name="srowp", bufs=4) as srp, \
             tc.tile_pool(name="rinvp", bufs=2) as rip, \
             tc.tile_pool(name="ytp", bufs=2) as ytp:
            for u in proj_units(0):
                u()
            # qj 0: short attention, then the n=1 projections as one burst
            attn_unit(0, 0)
            attn_unit(0, 1)
            for u in proj_units(1):
                u()
            # qj 1..3: interleave next block's projections + prev outproj
            for qj in range(1, NT):
                fill = proj_units(qj + 1) if qj + 1 < NT else []
                attn_unit(qj, 0)
                for u in fill[:4]:
                    u()
                attn_unit(qj, 1)
                for u in fill[4:]:
                    u()
                outproj(qj - 1)
            outproj(NT - 1, last=True)

    _split_excess_waits(nc)
    nc.compile()
    return nc


def _split_excess_waits(nc):
    """Walrus caps most instructions at 1 sync wait. Peel excess waits off
    matmuls (and anything else over the cap) onto PE-engine wait-nops
    inserted immediately before the instruction."""
    for bb in nc.main_func.blocks:
        new_insts = []
        for inst in bb.instructions:
            si = inst.sync_info
            if (si is not None and si.on_wait and len(si.on_wait) > 1
                    and isinstance(inst, mybir.InstMatmult)):
                excess = list(si.on_wait[:-1])
                keep = [si.on_wait[-1]]
                for w in excess:
                    nop = mybir.InstNoOp(
                        name=nc.get_next_instruction_name(), ins=[], outs=[],
                        bass_nofuse=True)
                    nop.engine = inst.engine
                    nop.sync_info = mybir.SyncInfo(on_wait=[w], on_update=[])
                    nc.register_instruction(nop)
                    new_insts.append(nop)
                si.on_wait = keep
            new_insts.append(inst)
        bb.instructions[:] = new_insts


def _host_prep(x, Wq, Wkv, Wout, mask):
    bf16 = ml_dtypes.bfloat16
    x = np.asarray(x, dtype=np.float32)
    Wq = np.asarray(Wq, dtype=np.float32)
    Wkv = np.asarray(Wkv, dtype=np.float32)
    Wout = np.asarray(Wout, dtype=np.float32)
    mask = np.asarray(mask)

    xT = [np.ascontiguousarray(x[b].T).astype(bf16) for b in range(B)]
    keep = np.empty((128, T), dtype=np.float32)
    for i in range(T // 128):
        blk = mask[128 * i:128 * (i + 1), 128 * i:128 * (i + 1)]
        keep[:, 128 * i:128 * (i + 1)] = (~blk).T.astype(np.float32)
    keep = keep.astype(bf16)

    in_maps = []
    for core in range(NCORES):
        b, g = core // G, core % G
        sl = slice(DG * g, DG * (g + 1))
        in_maps.append({
            "xt": xT[b],
            "wq": np.ascontiguousarray(Wq[sl, :].T).astype(bf16),
            "wk": np.ascontiguousarray(Wkv[sl, :].T).astype(bf16),
            "wv": np.ascontiguousarray(Wkv[C + DG * g:C + DG * (g + 1), :].T).astype(bf16),
            "wo": np.ascontiguousarray(Wout[:, sl].T).astype(bf16),
            "keep": keep,
        })
    return in_maps


def _install_ntff_hook():
    import types
    import antenv
    if getattr(antenv, "axon_hooks", None) is not None:
        return
    ah = types.ModuleType("antenv.axon_hooks")
    ah._hook = None
    ah.set_axon_ntff_profile_hook = lambda h: setattr(ah, "_hook", h)
    ah.get_axon_ntff_profile_hook = lambda: ah._hook
    sys.modules["antenv.axon_hooks"] = ah
    antenv.axon_hooks = ah
    if "/root/.axon_site" not in sys.path:
        sys.path.insert(0, "/root/.axon_site")
    from trn_agent_boot.trn_boot import _ntff_profile_via_ctypes
    ah.set_axon_ntff_profile_hook(_ntff_profile_via_ctypes("/opt/axon/libaxon_pjrt.so"))


def _run(inputs, trace=False):
    global _cached_nc
    from concourse.bass_utils import run_bass_kernel_spmd
    if trace:
        _install_ntff_hook()
    if _cached_nc is None:
        _cached_nc = build_nc()
    in_maps = _host_prep(**inputs)
    res = run_bass_kernel_spmd(_cached_nc, in_maps, list(range(NCORES)), trace=trace)
    parts = [np.asarray(res.results[c]["y"], dtype=np.float32) for c in range(NCORES)]
    out = np.stack([
        parts[0] + parts[1] + parts[2] + parts[3],
        parts[4] + parts[5] + parts[6] + parts[7],
    ]).astype(np.float32)
    return out, res


def kernel(x, Wq, Wkv, Wout, mask):
    out, _ = _run(dict(x=x, Wq=Wq, Wkv=Wkv, Wout=Wout, mask=mask))
    return out


# revision 13
# speedup vs baseline: 2.0265x; 1.0376x over previous
"""Trainium2 Bass kernel for causal MHA (B=2, T=2048, D=1024, H=16, KH=64).

Sharding: 8 cores = 2 (batch) x 4 (head groups of 4 heads).
Each core computes q/k/v projections for its 4 heads, causal attention,
and a partial output projection against its 256-row slice of Wout.
Host sums the 4 partials per batch (the all-reduce step, done at unshard).

v5: bf16 matmul pipeline end to end; q/k/v projection chains interleaved
into the attention stream as PE filler (the attention stretch is paced by
the scalar-engine EXP chain, so a separate projection phase both serializes
the walls and lets the PE clock-gate re-throttle); fast approximate
reciprocal on the matmul-broadcast denominator; pair-batched EXP; output
projection of block qj-1 riding its own PSUM slots inside block qj.
"""
import sys

sys.path.insert(0, "/opt/trn_rl_repo")

from contextlib import ExitStack

import numpy as np
import ml_dtypes

import concourse.bacc as bacc
import concourse.mybir as mybir
import concourse.tile as tile

B, T, C = 2, 2048, 1024
H, KH = 16, 64
G = 4                 # head groups
HPG = H // G          # heads per group = 4
DG = HPG * KH         # 256 per-core head dims
NCORES = 8

F32 = mybir.dt.float32
F32R = mybir.dt.float32r
BF16 = mybir.dt.bfloat16
EXP = mybir.ActivationFunctionType.Exp
COPY = mybir.ActivationFunctionType.Copy

_cached_nc = None


def build_nc():
    nc = bacc.Bacc()
    xt = nc.dram_tensor("xt", [C, T], BF16, kind="ExternalInput")        # x[b].T
    wq = nc.dram_tensor("wq", [C, DG], BF16, kind="ExternalInput")       # Wq slice .T
    wk = nc.dram_tensor("wk", [C, DG], BF16, kind="ExternalInput")
    wv = nc.dram_tensor("wv", [C, DG], BF16, kind="ExternalInput")
    wo = nc.dram_tensor("wo", [DG, C], BF16, kind="ExternalInput")       # Wout[:, slice].T
    keep = nc.dram_tensor("keep", [128, T], BF16, kind="ExternalInput")  # diag keep (k, q)
    y = nc.dram_tensor("y", [T, C], BF16, kind="ExternalOutput")         # partial output

    NT = T // 512     # 4 t blocks
    NK = C // 128     # 8 contraction chunks
    NTT = T // 128    # 16 t tiles of 128

    with ExitStack() as ctx:
        ctx.enter_context(nc.allow_low_precision(reason="bf16 matmul pipeline"))
        tc = ctx.enter_context(tile.TileContext(nc))
        persist = ctx.enter_context(tc.tile_pool(name="persist", bufs=1))
        psum = ctx.enter_context(tc.tile_pool(name="psum", bufs=2, space="PSUM"))

        # ---- persistent tiles ----
        qT = [persist.tile([128, T], BF16, tag=f"qT{i}", name=f"qT{i}") for i in range(2)]
        kT = [persist.tile([128, T], BF16, tag=f"kT{i}", name=f"kT{i}") for i in range(2)]
        vsb = [persist.tile([128, HPG, KH + 1], BF16, tag=f"v{i}", name=f"v{i}")
               for i in range(NTT)]
        aT2 = [persist.tile([128, T], BF16, tag=f"aT2{g}", name=f"aT2{g}")
               for g in range(2)]
        wo2 = [persist.tile([128, C], BF16, tag=f"wo2{g}", name=f"wo2{g}")
               for g in range(2)]
        keep_sb = persist.tile([128, T], BF16, tag="keep")
        ones_r = persist.tile([65, 64], F32R, tag="ones_r")
        ones_f32 = persist.tile([65, 64], F32, tag="ones_f32")
        xT = persist.tile([128, NK, T], BF16, tag="xT", name="xT")
        wq_sb = persist.tile([128, NK, DG], BF16, tag="wq_sb", name="wq_sb")
        wk_sb = persist.tile([128, NK, DG], BF16, tag="wk_sb", name="wk_sb")
        wv_sb = persist.tile([128, NK, DG], BF16, tag="wv_sb", name="wv_sb")

        # DMA issue order = first-use order. Split the head of the x load so
        # the first matmul group is fed as early as possible.
        xt_r = xt.rearrange("(k p) t -> p k t", p=128)
        wq_r = wq.rearrange("(k p) d -> p k d", p=128)
        nc.sync.dma_start(out=wq_sb[:, :, 0:128], in_=wq_r[:, :, 0:128])
        nc.sync.dma_start(out=xT[:, 0:4, 0:512], in_=xt_r[:, 0:4, 0:512])
        nc.sync.dma_start(out=xT[:, 4:8, 0:512], in_=xt_r[:, 4:8, 0:512])
        nc.sync.dma_start(out=wq_sb[:, :, 128:256], in_=wq_r[:, :, 128:256])
        nc.sync.dma_start(out=wk_sb, in_=wk.rearrange("(k p) d -> p k d", p=128))
        nc.sync.dma_start(out=keep_sb, in_=keep[:, :])
        nc.sync.dma_start(out=wv_sb, in_=wv.rearrange("(k p) d -> p k d", p=128))
        for n in range(1, NT):
            nc.sync.dma_start(out=xT[:, :, n * 512:(n + 1) * 512],
                              in_=xt_r[:, :, n * 512:(n + 1) * 512])
        for g in range(2):
            nc.sync.dma_start(out=wo2[g], in_=wo[g * 128:(g + 1) * 128, :])

        nc.vector.memset(ones_f32, 1.0)
        nc.vector.tensor_copy(out=ones_r, in_=ones_f32)
        for tt in range(NTT):
            nc.gpsimd.memset(vsb[tt][:, :, KH:KH + 1], 1.0)

        # ---- projection chains, decomposed into per-matmul filler thunks ----
        def qk_thunks(dst, w_sb, m, n):
            st = {}
            def mm(k):
                if k == 0:
                    st["ps"] = psum.tile([128, 512], F32, tag="pj", name="ps")
                nc.tensor.matmul(
                    st["ps"],
                    w_sb[:, k, m * 128:(m + 1) * 128],
                    xT[:, k, n * 512:(n + 1) * 512],
                    start=(k == 0), stop=(k == NK - 1),
                )
            def fin():
                nc.vector.tensor_copy(
                    out=dst[m][:, n * 512:(n + 1) * 512], in_=st["ps"])
            return [lambda k=k: mm(k) for k in range(NK)] + [fin]

        def v_thunks(tt):
            st = {}
            def mm(k):
                if k == 0:
                    st["ps"] = psum.tile([128, DG], F32, tag="pj", name="ps")
                nc.tensor.matmul(
                    st["ps"],
                    xT[:, k, tt * 128:(tt + 1) * 128],
                    wv_sb[:, k, :],
                    start=(k == 0), stop=(k == NK - 1),
                )
            def fin():
                nc.vector.tensor_copy(
                    out=vsb[tt][:, :, 0:KH],
                    in_=st["ps"][:].rearrange("p (h d) -> p h d", h=HPG),
                )
            return [lambda k=k: mm(k) for k in range(NK)] + [fin]

        def proj_thunks(n):
            th = []
            for dst, w_sb in ((qT, wq_sb), (kT, wk_sb)):
                for m in range(2):
                    th += qk_thunks(dst, w_sb, m, n)
            for i in range(4):
                th += v_thunks(4 * n + i)
            return th

        def outproj_thunks(qj, last=False):
            th = []
            for i in range(4):
                tt = 4 * qj + i
                st = {}
                def mk(tt, st, no, g, last):
                    def mm():
                        if no == 0 and g == 0:
                            st["yt"] = ytp.tile([128, C], BF16, tag="yt",
                                                name="yt")
                        if g == 0:
                            st["yp"] = psum.tile([128, 512], F32, tag="pj",
                                                 name="yp")
                        nc.tensor.matmul(
                            st["yp"],
                            aT2[g][:, tt * 128:(tt + 1) * 128],
                            wo2[g][:, no * 512:(no + 1) * 512],
                            start=(g == 0), stop=(g == 1),
                        )
                    def fin():
                        sl = slice(no * 512, (no + 1) * 512)
                        if last and no == 1:
                            nc.scalar.activation(
                                out=st["yt"][:, sl], in_=st["yp"], func=COPY)
                        else:
                            nc.vector.tensor_copy(
                                out=st["yt"][:, sl], in_=st["yp"])
                        nc.sync.dma_start(
                            out=y[tt * 128:(tt + 1) * 128, sl],
                            in_=st["yt"][:, sl])
                    return mm, fin
                for no in range(2):
                    for g in range(2):
                        mm, fin = mk(tt, st, no, g, last)
                        th.append(mm)
                        if g == 1:
                            th.append(fin)
            return th

        def attn_unit(qj, hp, fill=None):
            kmax = 4 * qj + 4
            acc = [psum.tile([65, 512], F32, tag=f"pv{par}", bufs=1,
                             name=f"pv{par}")
                   for par in range(2)]
            for kt in range(kmax):
                off = 128 * (kt - 4 * qj) if kt >= 4 * qj else 0
                sc2 = psum.tile([128, 1024], F32, tag="sc", name="sc2")
                for par in range(2):
                    nc.tensor.matmul(
                        sc2[:, par * 512 + off:par * 512 + 512],
                        kT[hp][64 * par:64 * par + 64, kt * 128:(kt + 1) * 128],
                        qT[hp][64 * par:64 * par + 64,
                               qj * 512 + off:(qj + 1) * 512],
                        start=True, stop=True,
                    )
                if fill:
                    fill.popleft()() if len(fill) else None
                pt2 = ptp.tile([128, 2, 512], BF16, tag="pt", name="pt2")
                nc.scalar.activation(
                    out=pt2[:, :, off:512],
                    in_=sc2[:].rearrange("p (two n) -> p two n", two=2)[:, :, off:512],
                    func=EXP, scale=0.125)
                if kt >= 4 * qj:
                    for par in range(2):
                        nc.gpsimd.tensor_mul(
                            pt2[:, par, off:off + 128],
                            pt2[:, par, off:off + 128],
                            keep_sb[:, kt * 128:(kt + 1) * 128],
                        )
                for par in range(2):
                    nc.tensor.matmul(
                        acc[par][:, off:512],
                        vsb[kt][:, 2 * hp + par, :],
                        pt2[:, par, off:512],
                        start=(kt == 0), stop=(kt == kmax - 1),
                    )
                if fill:
                    for _ in range(min(2, len(fill))):
                        fill.popleft()()
            for par in range(2):
                srow = srp.tile([65, 512], F32R, tag="srow", name="srow")
                nc.vector.tensor_copy(out=srow[64:65, :], in_=acc[par][64:65, :])
                rbc = psum.tile([64, 512], F32, tag="pj", name="rbc")
                nc.tensor.matmul(
                    rbc, ones_r[64:65, :], srow[64:65, :], start=True, stop=True)
                rinv = rip.tile([64, 512], F32, tag="rinv", name="rinv")
                nc.vector.reciprocal_approx_fast(out=rinv, in_=rbc)
                nc.vector.tensor_mul(
                    aT2[hp][64 * par:64 * par + 64, qj * 512:(qj + 1) * 512],
                    acc[par][0:64, :],
                    rinv,
                )

        from collections import deque

        with tc.tile_pool(name="pts", bufs=6) as ptp, \
             tc.tile_pool(name="srowp", bufs=4) as srp, \
             tc.tile_pool(name="rinvp", bufs=2) as rip, \
             tc.tile_pool(name="ytp", bufs=3) as ytp:
            def burst(th):
                for t in th:
                    t()
            burst(proj_thunks(0))
            # qj 0 region is PE-rich: short attention + n=1 projection burst
            attn_unit(0, 0)
            attn_unit(0, 1)
            burst(proj_thunks(1))
            # qj 1..2: drip next block's projections into the kt loop,
            # prev block's output projection at the boundary
            for qj in (1, 2):
                fill = deque(proj_thunks(qj + 1))
                attn_unit(qj, 0, fill)
                attn_unit(qj, 1, fill)
                burst(fill)
                burst(outproj_thunks(qj - 1))
            # qj 3: outproj(2) drips into hp0's kt loop
            fill = deque(outproj_thunks(2))
            attn_unit(3, 0, fill)
            burst(fill)
            attn_unit(3, 1)
            burst(outproj_thunks(3, last=True))

    _split_excess_waits(nc)
    nc.compile()
    return nc


def _split_excess_waits(nc):
    """Walrus caps most instructions at 1 sync wait. Peel excess waits off
    matmuls (and anything else over the cap) onto PE-engine wait-nops
    inserted immediately before the instruction."""
    for bb in nc.main_func.blocks:
        new_insts = []
        for inst in bb.instructions:
            si = inst.sync_info
            if (si is not None and si.on_wait and len(si.on_wait) > 1
                    and isinstance(inst, mybir.InstMatmult)):
                excess = list(si.on_wait[:-1])
                keep = [si.on_wait[-1]]
                for w in excess:
                    nop = mybir.InstNoOp(
                        name=nc.get_next_instruction_name(), ins=[], outs=[],
                        bass_nofuse=True)
                    nop.engine = inst.engine
                    nop.sync_info = mybir.SyncInfo(on_wait=[w], on_update=[])
                    nc.register_instruction(nop)
                    new_insts.append(nop)
                si.on_wait = keep
            new_insts.append(inst)
        bb.instructions[:] = new_insts


def _host_prep(x, Wq, Wkv, Wout, mask):
    bf16 = ml_dtypes.bfloat16
    x = np.asarray(x, dtype=np.float32)
    Wq = np.asarray(Wq, dtype=np.float32)
    Wkv = np.asarray(Wkv, dtype=np.float32)
    Wout = np.asarray(Wout, dtype=np.float32)
    mask = np.asarray(mask)

    xT = [np.ascontiguousarray(x[b].T).astype(bf16) for b in range(B)]
    keep = np.empty((128, T), dtype=np.float32)
    for i in range(T // 128):
        blk = mask[128 * i:128 * (i + 1), 128 * i:128 * (i + 1)]
        keep[:, 128 * i:128 * (i + 1)] = (~blk).T.astype(np.float32)
    keep = keep.astype(bf16)

    in_maps = []
    for core in range(NCORES):
        b, g = core // G, core % G
        sl = slice(DG * g, DG * (g + 1))
        in_maps.append({
            "xt": xT[b],
            "wq": np.ascontiguousarray(Wq[sl, :].T).astype(bf16),
            "wk": np.ascontiguousarray(Wkv[sl, :].T).astype(bf16),
            "wv": np.ascontiguousarray(Wkv[C + DG * g:C + DG * (g + 1), :].T).astype(bf16),
            "wo": np.ascontiguousarray(Wout[:, sl].T).astype(bf16),
            "keep": keep,
        })
    return in_maps


def _install_ntff_hook():
    import types
    import antenv
    if getattr(antenv, "axon_hooks", None) is not None:
        return
    ah = types.ModuleType("antenv.axon_hooks")
    ah._hook = None
    ah.set_axon_ntff_profile_hook = lambda h: setattr(ah, "_hook", h)
    ah.get_axon_ntff_profile_hook = lambda: ah._hook
    sys.modules["antenv.axon_hooks"] = ah
    antenv.axon_hooks = ah
    if "/root/.axon_site" not in sys.path:
        sys.path.insert(0, "/root/.axon_site")
    from trn_agent_boot.trn_boot import _ntff_profile_via_ctypes
    ah.set_axon_ntff_profile_hook(_ntff_profile_via_ctypes("/opt/axon/libaxon_pjrt.so"))


def _run(inputs, trace=False):
    global _cached_nc
    from concourse.bass_utils import run_bass_kernel_spmd
    if trace:
        _install_ntff_hook()
    if _cached_nc is None:
        _cached_nc = build_nc()
    in_maps = _host_prep(**inputs)
    res = run_bass_kernel_spmd(_cached_nc, in_maps, list(range(NCORES)), trace=trace)
    parts = [np.asarray(res.results[c]["y"], dtype=np.float32) for c in range(NCORES)]
    out = np.stack([
        parts[0] + parts[1] + parts[2] + parts[3],
        parts[4] + parts[5] + parts[6] + parts[7],
    ]).astype(np.float32)
    return out, res


def kernel(x, Wq, Wkv, Wout, mask):
    out, _ = _run(dict(x=x, Wq=Wq, Wkv=Wkv, Wout=Wout, mask=mask))
    return out


# revision 16
# speedup vs baseline: 2.1659x; 1.0688x over previous
"""Trainium2 Bass kernel for causal MHA (B=2, T=2048, D=1024, H=16, KH=64).

Sharding: 8 cores = 2 (batch) x 4 (head groups of 4 heads).
Each core computes q/k/v projections for its 4 heads, causal attention,
and a partial output projection against its 256-row slice of Wout.
Host sums the 4 partials per batch (the all-reduce step, done at unshard).

v5: bf16 matmul pipeline end to end; q/k/v projection chains interleaved
into the attention stream as PE filler (the attention stretch is paced by
the scalar-engine EXP chain, so a separate projection phase both serializes
the walls and lets the PE clock-gate re-throttle); fast approximate
reciprocal on the matmul-broadcast denominator; pair-batched EXP; output
projection of block qj-1 riding its own PSUM slots inside block qj.
"""
import sys

sys.path.insert(0, "/opt/trn_rl_repo")

from contextlib import ExitStack

import numpy as np
import ml_dtypes

import concourse.bacc as bacc
import concourse.mybir as mybir
import concourse.tile as tile

B, T, C = 2, 2048, 1024
H, KH = 16, 64
G = 4                 # head groups
HPG = H // G          # heads per group = 4
DG = HPG * KH         # 256 per-core head dims
NCORES = 8

F32 = mybir.dt.float32
F32R = mybir.dt.float32r
BF16 = mybir.dt.bfloat16
EXP = mybir.ActivationFunctionType.Exp
COPY = mybir.ActivationFunctionType.Copy

_cached_nc = None


def build_nc():
    nc = bacc.Bacc()
    xt = nc.dram_tensor("xt", [C, T], BF16, kind="ExternalInput")        # x[b].T
    wq = nc.dram_tensor("wq", [C, DG], BF16, kind="ExternalInput")       # Wq slice .T
    wk = nc.dram_tensor("wk", [C, DG], BF16, kind="ExternalInput")
    wv = nc.dram_tensor("wv", [C, DG], BF16, kind="ExternalInput")
    wo = nc.dram_tensor("wo", [DG, C], BF16, kind="ExternalInput")       # Wout[:, slice].T
    keep = nc.dram_tensor("keep", [128, T], BF16, kind="ExternalInput")  # diag keep (k, q)
    y = nc.dram_tensor("y", [T, C], BF16, kind="ExternalOutput")         # partial output

    NT = T // 512     # 4 t blocks
    NK = C // 128     # 8 contraction chunks
    NTT = T // 128    # 16 t tiles of 128

    with ExitStack() as ctx:
        ctx.enter_context(nc.allow_low_precision(reason="bf16 matmul pipeline"))
        tc = ctx.enter_context(tile.TileContext(nc))
        persist = ctx.enter_context(tc.tile_pool(name="persist", bufs=1))
        psum = ctx.enter_context(tc.tile_pool(name="psum", bufs=2, space="PSUM"))

        # ---- persistent tiles ----
        qT = [persist.tile([128, T], BF16, tag=f"qT{i}", name=f"qT{i}") for i in range(2)]
        kT = [persist.tile([128, T], BF16, tag=f"kT{i}", name=f"kT{i}") for i in range(2)]
        vsb = [persist.tile([128, HPG, KH + 1], BF16, tag=f"v{i}", name=f"v{i}")
               for i in range(NTT)]
        aT2 = [persist.tile([128, T], BF16, tag=f"aT2{g}", name=f"aT2{g}")
               for g in range(2)]
        wo2 = [persist.tile([128, C], BF16, tag=f"wo2{g}", name=f"wo2{g}")
               for g in range(2)]
        keep_sb = persist.tile([128, T], BF16, tag="keep")
        ones_r = persist.tile([65, 64], F32R, tag="ones_r")
        ones_f32 = persist.tile([65, 64], F32, tag="ones_f32")
        xT = persist.tile([128, NK, T], BF16, tag="xT", name="xT")
        wq_sb = persist.tile([128, NK, DG], BF16, tag="wq_sb", name="wq_sb")
        wk_sb = persist.tile([128, NK, DG], BF16, tag="wk_sb", name="wk_sb")
        wv_sb = persist.tile([128, NK, DG], BF16, tag="wv_sb", name="wv_sb")

        # DMA issue order = first-use order. Split the head of the x load so
        # the first matmul group is fed as early as possible.
        xt_r = xt.rearrange("(k p) t -> p k t", p=128)
        wq_r = wq.rearrange("(k p) d -> p k d", p=128)
        nc.sync.dma_start(out=wq_sb[:, :, 0:128], in_=wq_r[:, :, 0:128])
        nc.sync.dma_start(out=xT[:, 0:4, 0:512], in_=xt_r[:, 0:4, 0:512])
        nc.sync.dma_start(out=xT[:, 4:8, 0:512], in_=xt_r[:, 4:8, 0:512])
        nc.sync.dma_start(out=wq_sb[:, :, 128:256], in_=wq_r[:, :, 128:256])
        nc.sync.dma_start(out=wk_sb, in_=wk.rearrange("(k p) d -> p k d", p=128))
        nc.sync.dma_start(out=keep_sb, in_=keep[:, :])
        nc.sync.dma_start(out=wv_sb, in_=wv.rearrange("(k p) d -> p k d", p=128))
        for n in range(1, NT):
            nc.sync.dma_start(out=xT[:, :, n * 512:(n + 1) * 512],
                              in_=xt_r[:, :, n * 512:(n + 1) * 512])
        for g in range(2):
            nc.sync.dma_start(out=wo2[g], in_=wo[g * 128:(g + 1) * 128, :])

        nc.vector.memset(ones_f32, 1.0)
        nc.vector.tensor_copy(out=ones_r, in_=ones_f32)
        for tt in range(NTT):
            nc.gpsimd.memset(vsb[tt][:, :, KH:KH + 1], 1.0)

        # ---- projection chains, decomposed into per-matmul filler thunks ----
        def qk_thunks(dst, w_sb, m, n):
            st = {}
            def mm(k):
                if k == 0:
                    st["ps"] = psum.tile([128, 512], F32, tag="pj", name="ps")
                nc.tensor.matmul(
                    st["ps"],
                    w_sb[:, k, m * 128:(m + 1) * 128],
                    xT[:, k, n * 512:(n + 1) * 512],
                    start=(k == 0), stop=(k == NK - 1),
                )
            def fin():
                nc.vector.tensor_copy(
                    out=dst[m][:, n * 512:(n + 1) * 512], in_=st["ps"])
            return [lambda k=k: mm(k) for k in range(NK)] + [fin]

        def v_thunks(tt):
            st = {}
            def mm(k):
                if k == 0:
                    st["ps"] = psum.tile([128, DG], F32, tag="pj", name="ps")
                nc.tensor.matmul(
                    st["ps"],
                    xT[:, k, tt * 128:(tt + 1) * 128],
                    wv_sb[:, k, :],
                    start=(k == 0), stop=(k == NK - 1),
                )
            def fin():
                nc.vector.tensor_copy(
                    out=vsb[tt][:, :, 0:KH],
                    in_=st["ps"][:].rearrange("p (h d) -> p h d", h=HPG),
                )
            return [lambda k=k: mm(k) for k in range(NK)] + [fin]

        def proj_thunks(n):
            th = []
            for dst, w_sb in ((qT, wq_sb), (kT, wk_sb)):
                for m in range(2):
                    th += qk_thunks(dst, w_sb, m, n)
            for i in range(4):
                th += v_thunks(4 * n + i)
            return th

        def outproj_thunks(qj, last=False):
            th = []
            for i in range(4):
                tt = 4 * qj + i
                st = {}
                def mk(tt, st, no, g, last):
                    def mm():
                        if no == 0 and g == 0:
                            st["yt"] = ytp.tile([128, C], BF16, tag="yt",
                                                name="yt")
                        if g == 0:
                            st["yp"] = psum.tile([128, 512], F32, tag="pj",
                                                 name="yp")
                        nc.tensor.matmul(
                            st["yp"],
                            aT2[g][:, tt * 128:(tt + 1) * 128],
                            wo2[g][:, no * 512:(no + 1) * 512],
                            start=(g == 0), stop=(g == 1),
                        )
                    def fin():
                        sl = slice(no * 512, (no + 1) * 512)
                        if last and no == 1:
                            nc.scalar.activation(
                                out=st["yt"][:, sl], in_=st["yp"], func=COPY)
                        else:
                            nc.vector.tensor_copy(
                                out=st["yt"][:, sl], in_=st["yp"])
                        nc.sync.dma_start(
                            out=y[tt * 128:(tt + 1) * 128, sl],
                            in_=st["yt"][:, sl])
                    return mm, fin
                for no in range(2):
                    for g in range(2):
                        mm, fin = mk(tt, st, no, g, last)
                        th.append(mm)
                        if g == 1:
                            th.append(fin)
            return th

        def attn_unit(qj, hp, fill=None):
            kmax = 4 * qj + 4
            acc = [psum.tile([65, 512], F32, tag=f"pv{par}", bufs=1,
                             name=f"pv{par}")
                   for par in range(2)]
            for kt in range(kmax):
                off = 128 * (kt - 4 * qj) if kt >= 4 * qj else 0
                sc2 = psum.tile([128, 1024], F32, tag="sc", name="sc2")
                for par in range(2):
                    nc.tensor.matmul(
                        sc2[:, par * 512 + off:par * 512 + 512],
                        kT[hp][64 * par:64 * par + 64, kt * 128:(kt + 1) * 128],
                        qT[hp][64 * par:64 * par + 64,
                               qj * 512 + off:(qj + 1) * 512],
                        start=True, stop=True,
                    )
                # spread filler thunks evenly over the remaining kt slots so
                # the PE never idles below the HAM activity threshold
                n_take = 0
                if fill:
                    n_take = min(4, -(-len(fill) // (kmax - kt)))
                if n_take:
                    fill.popleft()()
                pt2 = ptp.tile([128, 2, 512], BF16, tag="pt", name="pt2")
                nc.scalar.activation(
                    out=pt2[:, :, off:512],
                    in_=sc2[:].rearrange("p (two n) -> p two n", two=2)[:, :, off:512],
                    func=EXP, scale=0.125)
                if kt >= 4 * qj:
                    for par in range(2):
                        nc.gpsimd.tensor_mul(
                            pt2[:, par, off:off + 128],
                            pt2[:, par, off:off + 128],
                            keep_sb[:, kt * 128:(kt + 1) * 128],
                        )
                for par in range(2):
                    nc.tensor.matmul(
                        acc[par][:, off:512],
                        vsb[kt][:, 2 * hp + par, :],
                        pt2[:, par, off:512],
                        start=(kt == 0), stop=(kt == kmax - 1),
                    )
                for _ in range(min(n_take - 1, len(fill) if fill else 0)):
                    fill.popleft()()
            for par in range(2):
                srow = srp.tile([65, 512], F32R, tag="srow", name="srow")
                nc.vector.tensor_copy(out=srow[64:65, :], in_=acc[par][64:65, :])
                rbc = psum.tile([64, 512], F32, tag="pj", name="rbc")
                nc.tensor.matmul(
                    rbc, ones_r[64:65, :], srow[64:65, :], start=True, stop=True)
                rinv = rip.tile([64, 512], F32, tag="rinv", name="rinv")
                nc.vector.reciprocal_approx_fast(out=rinv, in_=rbc)
                nc.vector.tensor_mul(
                    aT2[hp][64 * par:64 * par + 64, qj * 512:(qj + 1) * 512],
                    acc[par][0:64, :],
                    rinv,
                )

        from collections import deque

        with tc.tile_pool(name="pts", bufs=6) as ptp, \
             tc.tile_pool(name="srowp", bufs=4) as srp, \
             tc.tile_pool(name="rinvp", bufs=2) as rip, \
             tc.tile_pool(name="ytp", bufs=3) as ytp:
            def burst(th):
                for t in th:
                    t()
            burst(proj_thunks(0))
            # qj 0: drip the head of the n=1 projections through the short
            # attention, burst the rest
            fill = deque(proj_thunks(1))
            attn_unit(0, 0, fill)
            attn_unit(0, 1, fill)
            burst(fill)
            # qj 1..2: drip next block's projections into the kt loop,
            # prev block's output projection at the boundary
            for qj in (1, 2):
                fill = deque(proj_thunks(qj + 1))
                attn_unit(qj, 0, fill)
                attn_unit(qj, 1, fill)
                burst(fill)
                burst(outproj_thunks(qj - 1))
            # qj 3: outproj(2) drips through both head-pair kt loops
            op2 = outproj_thunks(2)
            fill = deque(op2[:12])
            attn_unit(3, 0, fill)
            burst(fill)
            fill = deque(op2[12:])
            attn_unit(3, 1, fill)
            burst(fill)
            burst(outproj_thunks(3, last=True))

    _split_excess_waits(nc)
    nc.compile()
    return nc


def _split_excess_waits(nc):
    """Walrus caps most instructions at 1 sync wait. Peel excess waits off
    matmuls (and anything else over the cap) onto PE-engine wait-nops
    inserted immediately before the instruction."""
    for bb in nc.main_func.blocks:
        new_insts = []
        for inst in bb.instructions:
            si = inst.sync_info
            if (si is not None and si.on_wait and len(si.on_wait) > 1
                    and isinstance(inst, mybir.InstMatmult)):
                excess = list(si.on_wait[:-1])
                keep = [si.on_wait[-1]]
                for w in excess:
                    nop = mybir.InstNoOp(
                        name=nc.get_next_instruction_name(), ins=[], outs=[],
                        bass_nofuse=True)
                    nop.engine = inst.engine
                    nop.sync_info = mybir.SyncInfo(on_wait=[w], on_update=[])
                    nc.register_instruction(nop)
                    new_insts.append(nop)
                si.on_wait = keep
            new_insts.append(inst)
        bb.instructions[:] = new_insts


def _host_prep(x, Wq, Wkv, Wout, mask):
    bf16 = ml_dtypes.bfloat16
    x = np.asarray(x, dtype=np.float32)
    Wq = np.asarray(Wq, dtype=np.float32)
    Wkv = np.asarray(Wkv, dtype=np.float32)
    Wout = np.asarray(Wout, dtype=np.float32)
    mask = np.asarray(mask)

    xT = [np.ascontiguousarray(x[b].T).astype(bf16) for b in range(B)]
    keep = np.empty((128, T), dtype=np.float32)
    for i in range(T // 128):
        blk = mask[128 * i:128 * (i + 1), 128 * i:128 * (i + 1)]
        keep[:, 128 * i:128 * (i + 1)] = (~blk).T.astype(np.float32)
    keep = keep.astype(bf16)

    in_maps = []
    for core in range(NCORES):
        b, g = core // G, core % G
        sl = slice(DG * g, DG * (g + 1))
        in_maps.append({
            "xt": xT[b],
            "wq": np.ascontiguousarray(Wq[sl, :].T).astype(bf16),
            "wk": np.ascontiguousarray(Wkv[sl, :].T).astype(bf16),
            "wv": np.ascontiguousarray(Wkv[C + DG * g:C + DG * (g + 1), :].T).astype(bf16),
            "wo": np.ascontiguousarray(Wout[:, sl].T).astype(bf16),
            "keep": keep,
        })
    return in_maps


def _install_ntff_hook():
    import types
    import antenv
    if getattr(antenv, "axon_hooks", None) is not None:
        return
    ah = types.ModuleType("antenv.axon_hooks")
    ah._hook = None
    ah.set_axon_ntff_profile_hook = lambda h: setattr(ah, "_hook", h)
    ah.get_axon_ntff_profile_hook = lambda: ah._hook
    sys.modules["antenv.axon_hooks"] = ah
    antenv.axon_hooks = ah
    if "/root/.axon_site" not in sys.path:
        sys.path.insert(0, "/root/.axon_site")
    from trn_agent_boot.trn_boot import _ntff_profile_via_ctypes
    ah.set_axon_ntff_profile_hook(_ntff_profile_via_ctypes("/opt/axon/libaxon_pjrt.so"))


def _run(inputs, trace=False):
    global _cached_nc
    from concourse.bass_utils import run_bass_kernel_spmd
    if trace:
        _install_ntff_hook()
    if _cached_nc is None:
        _cached_nc = build_nc()
    in_maps = _host_prep(**inputs)
    res = run_bass_kernel_spmd(_cached_nc, in_maps, list(range(NCORES)), trace=trace)
    parts = [np.asarray(res.results[c]["y"], dtype=np.float32) for c in range(NCORES)]
    out = np.stack([
        parts[0] + parts[1] + parts[2] + parts[3],
        parts[4] + parts[5] + parts[6] + parts[7],
    ]).astype(np.float32)
    return out, res


def kernel(x, Wq, Wkv, Wout, mask):
    out, _ = _run(dict(x=x, Wq=Wq, Wkv=Wkv, Wout=Wout, mask=mask))
    return out


# revision 20
# speedup vs baseline: 2.1713x; 1.0025x over previous
"""Trainium2 Bass kernel for causal MHA (B=2, T=2048, D=1024, H=16, KH=64).

Sharding: 8 cores = 2 (batch) x 4 (head groups of 4 heads).
Each core computes q/k/v projections for its 4 heads, causal attention,
and a partial output projection against its 256-row slice of Wout.
Host sums the 4 partials per batch (the all-reduce step, done at unshard).

v5: bf16 matmul pipeline end to end; q/k/v projection chains interleaved
into the attention stream as PE filler (the attention stretch is paced by
the scalar-engine EXP chain, so a separate projection phase both serializes
the walls and lets the PE clock-gate re-throttle); fast approximate
reciprocal on the matmul-broadcast denominator; pair-batched EXP; output
projection of block qj-1 riding its own PSUM slots inside block qj.
"""
import sys

sys.path.insert(0, "/opt/trn_rl_repo")

from contextlib import ExitStack

import numpy as np
import ml_dtypes

import concourse.bacc as bacc
import concourse.mybir as mybir
import concourse.tile as tile

B, T, C = 2, 2048, 1024
H, KH = 16, 64
G = 4                 # head groups
HPG = H // G          # heads per group = 4
DG = HPG * KH         # 256 per-core head dims
NCORES = 8

F32 = mybir.dt.float32
F32R = mybir.dt.float32r
BF16 = mybir.dt.bfloat16
EXP = mybir.ActivationFunctionType.Exp
COPY = mybir.ActivationFunctionType.Copy

_cached_nc = None


def build_nc():
    nc = bacc.Bacc()
    # x / weight inputs are host-swizzled to partition-major [128, k, ...]
    # so every DMA run is contiguous per partition.
    xt = nc.dram_tensor("xt", [128, C // 128, T], BF16, kind="ExternalInput")
    wq = nc.dram_tensor("wq", [128, C // 128, DG], BF16, kind="ExternalInput")
    wk = nc.dram_tensor("wk", [128, C // 128, DG], BF16, kind="ExternalInput")
    wv = nc.dram_tensor("wv", [128, C // 128, DG], BF16, kind="ExternalInput")
    wo = nc.dram_tensor("wo", [DG, C], BF16, kind="ExternalInput")       # Wout[:, slice].T
    keep = nc.dram_tensor("keep", [128, T], BF16, kind="ExternalInput")  # diag keep (k, q)
    y = nc.dram_tensor("y", [T, C], BF16, kind="ExternalOutput")         # partial output

    NT = T // 512     # 4 t blocks
    NK = C // 128     # 8 contraction chunks
    NTT = T // 128    # 16 t tiles of 128

    with ExitStack() as ctx:
        ctx.enter_context(nc.allow_low_precision(reason="bf16 matmul pipeline"))
        tc = ctx.enter_context(tile.TileContext(nc))
        persist = ctx.enter_context(tc.tile_pool(name="persist", bufs=1))
        psum = ctx.enter_context(tc.tile_pool(name="psum", bufs=2, space="PSUM"))

        # ---- persistent tiles ----
        qT = [persist.tile([128, T], BF16, tag=f"qT{i}", name=f"qT{i}") for i in range(2)]
        kT = [persist.tile([128, T], BF16, tag=f"kT{i}", name=f"kT{i}") for i in range(2)]
        vsb = [persist.tile([128, HPG, KH + 1], BF16, tag=f"v{i}", name=f"v{i}")
               for i in range(NTT)]
        aT2 = [persist.tile([128, T], BF16, tag=f"aT2{g}", name=f"aT2{g}")
               for g in range(2)]
        wo2 = [persist.tile([128, C], BF16, tag=f"wo2{g}", name=f"wo2{g}")
               for g in range(2)]
        keep_sb = persist.tile([128, T], BF16, tag="keep")
        ones_r = persist.tile([65, 64], F32R, tag="ones_r")
        ones_f32 = persist.tile([65, 64], F32, tag="ones_f32")
        xT = persist.tile([128, NK, T], BF16, tag="xT", name="xT")
        wq_sb = persist.tile([128, NK, DG], BF16, tag="wq_sb", name="wq_sb")
        wk_sb = persist.tile([128, NK, DG], BF16, tag="wk_sb", name="wk_sb")
        wv_sb = persist.tile([128, NK, DG], BF16, tag="wv_sb", name="wv_sb")

        # DMA issue order = first-use order. Split the head of the x load so
        # the first matmul group is fed as early as possible.
        nc.sync.dma_start(out=wq_sb, in_=wq[:, :, :])
        nc.sync.dma_start(out=xT[:, 0:4, 0:512], in_=xt[:, 0:4, 0:512])
        nc.sync.dma_start(out=xT[:, 4:8, 0:512], in_=xt[:, 4:8, 0:512])
        nc.sync.dma_start(out=wk_sb, in_=wk[:, :, :])
        nc.sync.dma_start(out=keep_sb, in_=keep[:, :])
        nc.sync.dma_start(out=wv_sb, in_=wv[:, :, :])
        for n in range(1, NT):
            nc.sync.dma_start(out=xT[:, :, n * 512:(n + 1) * 512],
                              in_=xt[:, :, n * 512:(n + 1) * 512])
        for g in range(2):
            nc.sync.dma_start(out=wo2[g], in_=wo[g * 128:(g + 1) * 128, :])

        nc.vector.memset(ones_f32, 1.0)
        nc.vector.tensor_copy(out=ones_r, in_=ones_f32)
        for tt in range(NTT):
            nc.gpsimd.memset(vsb[tt][:, :, KH:KH + 1], 1.0)

        # ---- projection chains, decomposed into per-matmul filler thunks ----
        def qk_thunks(dst, w_sb, m, n):
            st = {}
            def mm(k):
                if k == 0:
                    st["ps"] = psum.tile([128, 512], F32, tag="pj", name="ps")
                nc.tensor.matmul(
                    st["ps"],
                    w_sb[:, k, m * 128:(m + 1) * 128],
                    xT[:, k, n * 512:(n + 1) * 512],
                    start=(k == 0), stop=(k == NK - 1),
                )
            def fin():
                nc.vector.tensor_copy(
                    out=dst[m][:, n * 512:(n + 1) * 512], in_=st["ps"])
            return [lambda k=k: mm(k) for k in range(NK)] + [fin]

        def v_thunks(tt):
            st = {}
            def mm(k):
                if k == 0:
                    st["ps"] = psum.tile([128, DG], F32, tag="pj", name="ps")
                nc.tensor.matmul(
                    st["ps"],
                    xT[:, k, tt * 128:(tt + 1) * 128],
                    wv_sb[:, k, :],
                    start=(k == 0), stop=(k == NK - 1),
                )
            def fin():
                nc.vector.tensor_copy(
                    out=vsb[tt][:, :, 0:KH],
                    in_=st["ps"][:].rearrange("p (h d) -> p h d", h=HPG),
                )
            return [lambda k=k: mm(k) for k in range(NK)] + [fin]

        def proj_thunks(n):
            th = []
            for dst, w_sb in ((qT, wq_sb), (kT, wk_sb)):
                for m in range(2):
                    th += qk_thunks(dst, w_sb, m, n)
            for i in range(4):
                th += v_thunks(4 * n + i)
            return th

        def outproj_thunks(qj, last=False):
            th = []
            for i in range(4):
                tt = 4 * qj + i
                st = {}
                def mk(tt, st, no, g, last):
                    def mm():
                        if no == 0 and g == 0:
                            st["yt"] = ytp.tile([128, C], BF16, tag="yt",
                                                name="yt")
                        if g == 0:
                            st["yp"] = psum.tile([128, 512], F32, tag="pj",
                                                 name="yp")
                        nc.tensor.matmul(
                            st["yp"],
                            aT2[g][:, tt * 128:(tt + 1) * 128],
                            wo2[g][:, no * 512:(no + 1) * 512],
                            start=(g == 0), stop=(g == 1),
                        )
                    def fin():
                        sl = slice(no * 512, (no + 1) * 512)
                        if last and no == 1:
                            nc.scalar.activation(
                                out=st["yt"][:, sl], in_=st["yp"], func=COPY)
                        else:
                            nc.vector.tensor_copy(
                                out=st["yt"][:, sl], in_=st["yp"])
                        nc.sync.dma_start(
                            out=y[tt * 128:(tt + 1) * 128, sl],
                            in_=st["yt"][:, sl])
                    return mm, fin
                for no in range(2):
                    for g in range(2):
                        mm, fin = mk(tt, st, no, g, last)
                        th.append(mm)
                        if g == 1:
                            th.append(fin)
            return th

        def attn_unit(qj, hp, fill=None):
            kmax = 4 * qj + 4
            acc = [psum.tile([65, 512], F32, tag=f"pv{par}", bufs=1,
                             name=f"pv{par}")
                   for par in range(2)]
            for kt in range(kmax):
                off = 128 * (kt - 4 * qj) if kt >= 4 * qj else 0
                sc2 = psum.tile([128, 1024], F32, tag="sc", name="sc2")
                for par in range(2):
                    nc.tensor.matmul(
                        sc2[:, par * 512 + off:par * 512 + 512],
                        kT[hp][64 * par:64 * par + 64, kt * 128:(kt + 1) * 128],
                        qT[hp][64 * par:64 * par + 64,
                               qj * 512 + off:(qj + 1) * 512],
                        start=True, stop=True,
                    )
                # spread filler thunks evenly over the remaining kt slots so
                # the PE never idles below the HAM activity threshold
                n_take = 0
                if fill:
                    n_take = min(4, -(-len(fill) // (kmax - kt)))
                if n_take:
                    fill.popleft()()
                pt2 = ptp.tile([128, 2, 512], BF16, tag="pt", name="pt2")
                nc.scalar.activation(
                    out=pt2[:, :, off:512],
                    in_=sc2[:].rearrange("p (two n) -> p two n", two=2)[:, :, off:512],
                    func=EXP, scale=0.125)
                if kt >= 4 * qj:
                    for par in range(2):
                        nc.gpsimd.tensor_mul(
                            pt2[:, par, off:off + 128],
                            pt2[:, par, off:off + 128],
                            keep_sb[:, kt * 128:(kt + 1) * 128],
                        )
                for par in range(2):
                    nc.tensor.matmul(
                        acc[par][:, off:512],
                        vsb[kt][:, 2 * hp + par, :],
                        pt2[:, par, off:512],
                        start=(kt == 0), stop=(kt == kmax - 1),
                    )
                for _ in range(min(n_take - 1, len(fill) if fill else 0)):
                    fill.popleft()()
            for par in range(2):
                srow = srp.tile([65, 512], F32R, tag="srow", name="srow")
                nc.vector.tensor_copy(out=srow[64:65, :], in_=acc[par][64:65, :])
                rbc = psum.tile([64, 512], F32, tag="pj", name="rbc")
                nc.tensor.matmul(
                    rbc, ones_r[64:65, :], srow[64:65, :], start=True, stop=True)
                rinv = rip.tile([64, 512], F32, tag="rinv", name="rinv")
                nc.vector.reciprocal_approx_fast(out=rinv, in_=rbc)
                nc.vector.tensor_mul(
                    aT2[hp][64 * par:64 * par + 64, qj * 512:(qj + 1) * 512],
                    acc[par][0:64, :],
                    rinv,
                )

        from collections import deque

        with tc.tile_pool(name="pts", bufs=6) as ptp, \
             tc.tile_pool(name="srowp", bufs=4) as srp, \
             tc.tile_pool(name="rinvp", bufs=2) as rip, \
             tc.tile_pool(name="ytp", bufs=3) as ytp:
            def burst(th):
                for t in th:
                    t()
            burst(proj_thunks(0))
            # qj 0: drip the head of the n=1 projections through the short
            # attention, burst the rest
            fill = deque(proj_thunks(1))
            attn_unit(0, 0, fill)
            attn_unit(0, 1, fill)
            burst(fill)
            # qj 1..2: drip next block's projections into the kt loop,
            # prev block's output projection at the boundary
            for qj in (1, 2):
                fill = deque(proj_thunks(qj + 1))
                attn_unit(qj, 0, fill)
                attn_unit(qj, 1, fill)
                burst(fill)
                burst(outproj_thunks(qj - 1))
            # qj 3: outproj(2) drips through both head-pair kt loops, with
            # the larger share in hp1 (the last long scalar-paced stretch)
            op2 = outproj_thunks(2)
            fill = deque(op2[:6])
            attn_unit(3, 0, fill)
            burst(fill)
            fill = deque(op2[6:])
            attn_unit(3, 1, fill)
            burst(fill)
            burst(outproj_thunks(3, last=True))

    _split_excess_waits(nc)
    nc.compile()
    return nc


def _split_excess_waits(nc):
    """Walrus caps most instructions at 1 sync wait. Peel excess waits off
    matmuls (and anything else over the cap) onto PE-engine wait-nops
    inserted immediately before the instruction."""
    for bb in nc.main_func.blocks:
        new_insts = []
        for inst in bb.instructions:
            si = inst.sync_info
            if (si is not None and si.on_wait and len(si.on_wait) > 1
                    and isinstance(inst, mybir.InstMatmult)):
                excess = list(si.on_wait[:-1])
                keep = [si.on_wait[-1]]
                for w in excess:
                    nop = mybir.InstNoOp(
                        name=nc.get_next_instruction_name(), ins=[], outs=[],
                        bass_nofuse=True)
                    nop.engine = inst.engine
                    nop.sync_info = mybir.SyncInfo(on_wait=[w], on_update=[])
                    nc.register_instruction(nop)
                    new_insts.append(nop)
                si.on_wait = keep
            new_insts.append(inst)
        bb.instructions[:] = new_insts


def _host_prep(x, Wq, Wkv, Wout, mask):
    bf16 = ml_dtypes.bfloat16
    x = np.asarray(x, dtype=np.float32)
    Wq = np.asarray(Wq, dtype=np.float32)
    Wkv = np.asarray(Wkv, dtype=np.float32)
    Wout = np.asarray(Wout, dtype=np.float32)
    mask = np.asarray(mask)

    def swiz(a):
        # [C, D] -> partition-major [128, C//128, D], contiguous bf16
        return np.ascontiguousarray(
            a.reshape(C // 128, 128, a.shape[1]).transpose(1, 0, 2)).astype(bf16)

    xT = [swiz(np.ascontiguousarray(x[b].T)) for b in range(B)]
    keep = np.empty((128, T), dtype=np.float32)
    for i in range(T // 128):
        blk = mask[128 * i:128 * (i + 1), 128 * i:128 * (i + 1)]
        keep[:, 128 * i:128 * (i + 1)] = (~blk).T.astype(np.float32)
    keep = keep.astype(bf16)

    in_maps = []
    for core in range(NCORES):
        b, g = core // G, core % G
        sl = slice(DG * g, DG * (g + 1))
        in_maps.append({
            "xt": xT[b],
            "wq": swiz(np.ascontiguousarray(Wq[sl, :].T)),
            "wk": swiz(np.ascontiguousarray(Wkv[sl, :].T)),
            "wv": swiz(np.ascontiguousarray(Wkv[C + DG * g:C + DG * (g + 1), :].T)),
            "wo": np.ascontiguousarray(Wout[:, sl].T).astype(bf16),
            "keep": keep,
        })
    return in_maps


def _install_ntff_hook():
    import types
    import antenv
    if getattr(antenv, "axon_hooks", None) is not None:
        return
    ah = types.ModuleType("antenv.axon_hooks")
    ah._hook = None
    ah.set_axon_ntff_profile_hook = lambda h: setattr(ah, "_hook", h)
    ah.get_axon_ntff_profile_hook = lambda: ah._hook
    sys.modules["antenv.axon_hooks"] = ah
    antenv.axon_hooks = ah
    if "/root/.axon_site" not in sys.path:
        sys.path.insert(0, "/root/.axon_site")
    from trn_agent_boot.trn_boot import _ntff_profile_via_ctypes
    ah.set_axon_ntff_profile_hook(_ntff_profile_via_ctypes("/opt/axon/libaxon_pjrt.so"))


def _run(inputs, trace=False):
    global _cached_nc
    from concourse.bass_utils import run_bass_kernel_spmd
    if trace:
        _install_ntff_hook()
    if _cached_nc is None:
        _cached_nc = build_nc()
    in_maps = _host_prep(**inputs)
    res = run_bass_kernel_spmd(_cached_nc, in_maps, list(range(NCORES)), trace=trace)
    parts = [np.asarray(res.results[c]["y"], dtype=np.float32) for c in range(NCORES)]
    out = np.stack([
        parts[0] + parts[1] + parts[2] + parts[3],
        parts[4] + parts[5] + parts[6] + parts[7],
    ]).astype(np.float32)
    return out, res


def kernel(x, Wq, Wkv, Wout, mask):
    out, _ = _run(dict(x=x, Wq=Wq, Wkv=Wkv, Wout=Wout, mask=mask))
    return out


# revision 21
# speedup vs baseline: 2.2556x; 1.0388x over previous
"""Trainium2 Bass kernel for causal MHA (B=2, T=2048, D=1024, H=16, KH=64).

Sharding: 8 cores = 2 (batch) x 4 (head groups of 4 heads).
Each core computes q/k/v projections for its 4 heads, causal attention,
and a partial output projection against its 256-row slice of Wout.
Host sums the 4 partials per batch (the all-reduce step, done at unshard).

v5: bf16 matmul pipeline end to end; q/k/v projection chains interleaved
into the attention stream as PE filler (the attention stretch is paced by
the scalar-engine EXP chain, so a separate projection phase both serializes
the walls and lets the PE clock-gate re-throttle); fast approximate
reciprocal on the matmul-broadcast denominator; pair-batched EXP; output
projection of block qj-1 riding its own PSUM slots inside block qj.
"""
import sys

sys.path.insert(0, "/opt/trn_rl_repo")

from contextlib import ExitStack

import numpy as np
import ml_dtypes

import concourse.bacc as bacc
import concourse.mybir as mybir
import concourse.tile as tile

B, T, C = 2, 2048, 1024
H, KH = 16, 64
G = 4                 # head groups
HPG = H // G          # heads per group = 4
DG = HPG * KH         # 256 per-core head dims
NCORES = 8

F32 = mybir.dt.float32
F32R = mybir.dt.float32r
BF16 = mybir.dt.bfloat16
EXP = mybir.ActivationFunctionType.Exp
COPY = mybir.ActivationFunctionType.Copy

_cached_nc = None


def build_nc():
    nc = bacc.Bacc()
    # x / weight inputs are host-swizzled to partition-major [128, k, ...]
    # so every DMA run is contiguous per partition.
    xt = nc.dram_tensor("xt", [128, C // 128, T], BF16, kind="ExternalInput")
    wq = nc.dram_tensor("wq", [128, C // 128, DG], BF16, kind="ExternalInput")
    wk = nc.dram_tensor("wk", [128, C // 128, DG], BF16, kind="ExternalInput")
    wv = nc.dram_tensor("wv", [128, C // 128, DG], BF16, kind="ExternalInput")
    wo = nc.dram_tensor("wo", [DG, C], BF16, kind="ExternalInput")       # Wout[:, slice].T
    keep = nc.dram_tensor("keep", [128, T], BF16, kind="ExternalInput")  # diag keep (k, q)
    y = nc.dram_tensor("y", [T, C], BF16, kind="ExternalOutput")         # partial output

    NT = T // 512     # 4 t blocks
    NK = C // 128     # 8 contraction chunks
    NTT = T // 128    # 16 t tiles of 128

    with ExitStack() as ctx:
        ctx.enter_context(nc.allow_low_precision(reason="bf16 matmul pipeline"))
        tc = ctx.enter_context(tile.TileContext(nc))
        persist = ctx.enter_context(tc.tile_pool(name="persist", bufs=1))
        psum = ctx.enter_context(tc.tile_pool(name="psum", bufs=2, space="PSUM"))

        # ---- persistent tiles ----
        qT = [persist.tile([128, T], BF16, tag=f"qT{i}", name=f"qT{i}") for i in range(2)]
        kT = [persist.tile([128, T], BF16, tag=f"kT{i}", name=f"kT{i}") for i in range(2)]
        vsb = [persist.tile([128, HPG, KH + 1], BF16, tag=f"v{i}", name=f"v{i}")
               for i in range(NTT)]
        aT2 = [persist.tile([128, T], BF16, tag=f"aT2{g}", name=f"aT2{g}")
               for g in range(2)]
        wo2 = [persist.tile([128, C], BF16, tag=f"wo2{g}", name=f"wo2{g}")
               for g in range(2)]
        keep_sb = persist.tile([128, T], BF16, tag="keep")
        ones_r = persist.tile([65, 64], F32R, tag="ones_r")
        ones_f32 = persist.tile([65, 64], F32, tag="ones_f32")
        xT = persist.tile([128, NK, T], BF16, tag="xT", name="xT")
        wq_sb = persist.tile([128, NK, DG], BF16, tag="wq_sb", name="wq_sb")
        wk_sb = persist.tile([128, NK, DG], BF16, tag="wk_sb", name="wk_sb")
        wv_sb = persist.tile([128, NK, DG], BF16, tag="wv_sb", name="wv_sb")

        # DMA issue order = first-use order. Split the head of the x load so
        # the first matmul group is fed as early as possible.
        nc.sync.dma_start(out=wq_sb, in_=wq[:, :, :])
        nc.sync.dma_start(out=xT[:, 0:4, 0:512], in_=xt[:, 0:4, 0:512])
        nc.sync.dma_start(out=xT[:, 4:8, 0:512], in_=xt[:, 4:8, 0:512])
        nc.sync.dma_start(out=wk_sb, in_=wk[:, :, :])
        nc.sync.dma_start(out=keep_sb, in_=keep[:, :])
        nc.sync.dma_start(out=wv_sb, in_=wv[:, :, :])
        for n in range(1, NT):
            nc.sync.dma_start(out=xT[:, :, n * 512:(n + 1) * 512],
                              in_=xt[:, :, n * 512:(n + 1) * 512])
        for g in range(2):
            nc.sync.dma_start(out=wo2[g], in_=wo[g * 128:(g + 1) * 128, :])

        nc.vector.memset(ones_f32, 1.0)
        nc.vector.tensor_copy(out=ones_r, in_=ones_f32)
        for tt in range(NTT):
            nc.gpsimd.memset(vsb[tt][:, :, KH:KH + 1], 1.0)

        # ---- projection chains, decomposed into per-matmul filler thunks ----
        def qk_thunks(dst, w_sb, m, n):
            st = {}
            def mm(k):
                if k == 0:
                    st["ps"] = psum.tile([128, 512], F32, tag="pj", name="ps")
                nc.tensor.matmul(
                    st["ps"],
                    w_sb[:, k, m * 128:(m + 1) * 128],
                    xT[:, k, n * 512:(n + 1) * 512],
                    start=(k == 0), stop=(k == NK - 1),
                )
            def fin():
                nc.vector.tensor_copy(
                    out=dst[m][:, n * 512:(n + 1) * 512], in_=st["ps"])
            return [lambda k=k: mm(k) for k in range(NK)] + [fin]

        def v_thunks(tt):
            st = {}
            def mm(k):
                if k == 0:
                    st["ps"] = psum.tile([128, DG], F32, tag="pj", name="ps")
                nc.tensor.matmul(
                    st["ps"],
                    xT[:, k, tt * 128:(tt + 1) * 128],
                    wv_sb[:, k, :],
                    start=(k == 0), stop=(k == NK - 1),
                )
            def fin():
                nc.vector.tensor_copy(
                    out=vsb[tt][:, :, 0:KH],
                    in_=st["ps"][:].rearrange("p (h d) -> p h d", h=HPG),
                )
            return [lambda k=k: mm(k) for k in range(NK)] + [fin]

        def proj_thunks(n):
            th = []
            for dst, w_sb in ((qT, wq_sb), (kT, wk_sb)):
                for m in range(2):
                    th += qk_thunks(dst, w_sb, m, n)
            for i in range(4):
                th += v_thunks(4 * n + i)
            return th

        def outproj_thunks(qj, last=False):
            th = []
            for i in range(4):
                tt = 4 * qj + i
                st = {}
                def mk(tt, st, no, g, last):
                    def mm():
                        if no == 0 and g == 0:
                            st["yt"] = ytp.tile([128, C], BF16, tag="yt",
                                                name="yt")
                        if g == 0:
                            st["yp"] = psum.tile([128, 512], F32, tag="pj",
                                                 name="yp")
                        nc.tensor.matmul(
                            st["yp"],
                            aT2[g][:, tt * 128:(tt + 1) * 128],
                            wo2[g][:, no * 512:(no + 1) * 512],
                            start=(g == 0), stop=(g == 1),
                        )
                    def fin():
                        sl = slice(no * 512, (no + 1) * 512)
                        if last and no == 1:
                            nc.scalar.activation(
                                out=st["yt"][:, sl], in_=st["yp"], func=COPY)
                        else:
                            nc.vector.tensor_copy(
                                out=st["yt"][:, sl], in_=st["yp"])
                        nc.sync.dma_start(
                            out=y[tt * 128:(tt + 1) * 128, sl],
                            in_=st["yt"][:, sl])
                    return mm, fin
                for no in range(2):
                    for g in range(2):
                        mm, fin = mk(tt, st, no, g, last)
                        th.append(mm)
                        if g == 1:
                            th.append(fin)
            return th

        def attn_unit(qj, hp, fill=None):
            kmax = 4 * qj + 4
            acc = [psum.tile([65, 512], F32, tag=f"pv{par}", bufs=1,
                             name=f"pv{par}")
                   for par in range(2)]
            for kt in range(kmax):
                off = 128 * (kt - 4 * qj) if kt >= 4 * qj else 0
                sc2 = psum.tile([128, 1024], F32, tag="sc", name="sc2")
                for par in range(2):
                    nc.tensor.matmul(
                        sc2[:, par * 512 + off:par * 512 + 512],
                        kT[hp][64 * par:64 * par + 64, kt * 128:(kt + 1) * 128],
                        qT[hp][64 * par:64 * par + 64,
                               qj * 512 + off:(qj + 1) * 512],
                        start=True, stop=True,
                    )
                # spread filler thunks evenly over the remaining kt slots so
                # the PE never idles below the HAM activity threshold
                n_take = 0
                if fill:
                    n_take = min(4, -(-len(fill) // (kmax - kt)))
                if n_take:
                    fill.popleft()()
                pt2 = ptp.tile([128, 2, 512], BF16, tag="pt", name="pt2")
                nc.scalar.activation(
                    out=pt2[:, :, off:512],
                    in_=sc2[:].rearrange("p (two n) -> p two n", two=2)[:, :, off:512],
                    func=EXP, scale=0.125)
                if kt >= 4 * qj:
                    for par in range(2):
                        nc.gpsimd.tensor_mul(
                            pt2[:, par, off:off + 128],
                            pt2[:, par, off:off + 128],
                            keep_sb[:, kt * 128:(kt + 1) * 128],
                        )
                for par in range(2):
                    nc.tensor.matmul(
                        acc[par][:, off:512],
                        vsb[kt][:, 2 * hp + par, :],
                        pt2[:, par, off:512],
                        start=(kt == 0), stop=(kt == kmax - 1),
                    )
                for _ in range(min(n_take - 1, len(fill) if fill else 0)):
                    fill.popleft()()
            for par in range(2):
                srow = srp.tile([65, 512], F32R, tag="srow", name="srow")
                nc.vector.tensor_copy(out=srow[64:65, :], in_=acc[par][64:65, :])
                rbc = psum.tile([64, 512], F32, tag="pj", name="rbc")
                nc.tensor.matmul(
                    rbc, ones_r[64:65, :], srow[64:65, :], start=True, stop=True)
                rinv = rip.tile([64, 512], F32, tag="rinv", name="rinv")
                nc.vector.reciprocal_approx_fast(out=rinv, in_=rbc)
                nc.vector.tensor_mul(
                    aT2[hp][64 * par:64 * par + 64, qj * 512:(qj + 1) * 512],
                    acc[par][0:64, :],
                    rinv,
                )

        from collections import deque

        with tc.tile_pool(name="pts", bufs=8) as ptp, \
             tc.tile_pool(name="srowp", bufs=6) as srp, \
             tc.tile_pool(name="rinvp", bufs=4) as rip, \
             tc.tile_pool(name="ytp", bufs=3) as ytp:
            def burst(th):
                for t in th:
                    t()
            burst(proj_thunks(0))
            # qj 0: drip the head of the n=1 projections through the short
            # attention, burst the rest
            fill = deque(proj_thunks(1))
            attn_unit(0, 0, fill)
            attn_unit(0, 1, fill)
            burst(fill)
            # qj 1..2: drip next block's projections into the kt loop.
            # All output projections are deferred into qj 3's long
            # scalar-paced stretches, which otherwise starve the PE.
            for qj in (1, 2):
                fill = deque(proj_thunks(qj + 1))
                attn_unit(qj, 0, fill)
                attn_unit(qj, 1, fill)
                burst(fill)
            fill = deque(outproj_thunks(0) + outproj_thunks(1))
            attn_unit(3, 0, fill)
            burst(fill)
            fill = deque(outproj_thunks(2))
            attn_unit(3, 1, fill)
            burst(fill)
            burst(outproj_thunks(3, last=True))

    _split_excess_waits(nc)
    nc.compile()
    return nc


def _split_excess_waits(nc):
    """Walrus caps most instructions at 1 sync wait. Peel excess waits off
    matmuls (and anything else over the cap) onto PE-engine wait-nops
    inserted immediately before the instruction."""
    for bb in nc.main_func.blocks:
        new_insts = []
        for inst in bb.instructions:
            si = inst.sync_info
            if (si is not None and si.on_wait and len(si.on_wait) > 1
                    and isinstance(inst, mybir.InstMatmult)):
                excess = list(si.on_wait[:-1])
                keep = [si.on_wait[-1]]
                for w in excess:
                    nop = mybir.InstNoOp(
                        name=nc.get_next_instruction_name(), ins=[], outs=[],
                        bass_nofuse=True)
                    nop.engine = inst.engine
                    nop.sync_info = mybir.SyncInfo(on_wait=[w], on_update=[])
                    nc.register_instruction(nop)
                    new_insts.append(nop)
                si.on_wait = keep
            new_insts.append(inst)
        bb.instructions[:] = new_insts


def _host_prep(x, Wq, Wkv, Wout, mask):
    bf16 = ml_dtypes.bfloat16
    x = np.asarray(x, dtype=np.float32)
    Wq = np.asarray(Wq, dtype=np.float32)
    Wkv = np.asarray(Wkv, dtype=np.float32)
    Wout = np.asarray(Wout, dtype=np.float32)
    mask = np.asarray(mask)

    def swiz(a):
        # [C, D] -> partition-major [128, C//128, D], contiguous bf16
        return np.ascontiguousarray(
            a.reshape(C // 128, 128, a.shape[1]).transpose(1, 0, 2)).astype(bf16)

    xT = [swiz(np.ascontiguousarray(x[b].T)) for b in range(B)]
    keep = np.empty((128, T), dtype=np.float32)
    for i in range(T // 128):
        blk = mask[128 * i:128 * (i + 1), 128 * i:128 * (i + 1)]
        keep[:, 128 * i:128 * (i + 1)] = (~blk).T.astype(np.float32)
    keep = keep.astype(bf16)

    in_maps = []
    for core in range(NCORES):
        b, g = core // G, core % G
        sl = slice(DG * g, DG * (g + 1))
        in_maps.append({
            "xt": xT[b],
            "wq": swiz(np.ascontiguousarray(Wq[sl, :].T)),
            "wk": swiz(np.ascontiguousarray(Wkv[sl, :].T)),
            "wv": swiz(np.ascontiguousarray(Wkv[C + DG * g:C + DG * (g + 1), :].T)),
            "wo": np.ascontiguousarray(Wout[:, sl].T).astype(bf16),
            "keep": keep,
        })
    return in_maps


def _install_ntff_hook():
    import types
    import antenv
    if getattr(antenv, "axon_hooks", None) is not None:
        return
    ah = types.ModuleType("antenv.axon_hooks")
    ah._hook = None
    ah.set_axon_ntff_profile_hook = lambda h: setattr(ah, "_hook", h)
    ah.get_axon_ntff_profile_hook = lambda: ah._hook
    sys.modules["antenv.axon_hooks"] = ah
    antenv.axon_hooks = ah
    if "/root/.axon_site" not in sys.path:
        sys.path.insert(0, "/root/.axon_site")
    from trn_agent_boot.trn_boot import _ntff_profile_via_ctypes
    ah.set_axon_ntff_profile_hook(_ntff_profile_via_ctypes("/opt/axon/libaxon_pjrt.so"))


def _run(inputs, trace=False):
    global _cached_nc
    from concourse.bass_utils import run_bass_kernel_spmd
    if trace:
        _install_ntff_hook()
    if _cached_nc is None:
        _cached_nc = build_nc()
    in_maps = _host_prep(**inputs)
    res = run_bass_kernel_spmd(_cached_nc, in_maps, list(range(NCORES)), trace=trace)
    parts = [np.asarray(res.results[c]["y"], dtype=np.float32) for c in range(NCORES)]
    out = np.stack([
        parts[0] + parts[1] + parts[2] + parts[3],
        parts[4] + parts[5] + parts[6] + parts[7],
    ]).astype(np.float32)
    return out, res


def kernel(x, Wq, Wkv, Wout, mask):
    out, _ = _run(dict(x=x, Wq=Wq, Wkv=Wkv, Wout=Wout, mask=mask))
    return out


# revision 29
# speedup vs baseline: 2.3115x; 1.0247x over previous
"""Trainium2 Bass kernel for causal MHA (B=2, T=2048, D=1024, H=16, KH=64).

Sharding: 8 cores = 2 (batch) x 4 (head groups of 4 heads).
Each core computes q/k/v projections for its 4 heads, causal attention,
and a partial output projection against its 256-row slice of Wout.
Host sums the 4 partials per batch (the all-reduce step, done at unshard).

v5: bf16 matmul pipeline end to end; q/k/v projection chains interleaved
into the attention stream as PE filler (the attention stretch is paced by
the scalar-engine EXP chain, so a separate projection phase both serializes
the walls and lets the PE clock-gate re-throttle); fast approximate
reciprocal on the matmul-broadcast denominator; pair-batched EXP; output
projection of block qj-1 riding its own PSUM slots inside block qj.
"""
import sys

sys.path.insert(0, "/opt/trn_rl_repo")

from contextlib import ExitStack

import numpy as np
import ml_dtypes

import concourse.bacc as bacc
import concourse.mybir as mybir
import concourse.tile as tile

B, T, C = 2, 2048, 1024
H, KH = 16, 64
G = 4                 # head groups
HPG = H // G          # heads per group = 4
DG = HPG * KH         # 256 per-core head dims
NCORES = 8

F32 = mybir.dt.float32
F32R = mybir.dt.float32r
BF16 = mybir.dt.bfloat16
EXP = mybir.ActivationFunctionType.Exp
COPY = mybir.ActivationFunctionType.Copy

_cached_nc = None


def build_nc():
    nc = bacc.Bacc()
    # x / weight inputs are host-swizzled to partition-major [128, k, ...]
    # so every DMA run is contiguous per partition.
    xt = nc.dram_tensor("xt", [128, C // 128, T], BF16, kind="ExternalInput")
    wq = nc.dram_tensor("wq", [128, 2, C // 128, 128], BF16, kind="ExternalInput")
    wk = nc.dram_tensor("wk", [128, 2, C // 128, 128], BF16, kind="ExternalInput")
    wv = nc.dram_tensor("wv", [128, C // 128, DG], BF16, kind="ExternalInput")
    wo = nc.dram_tensor("wo", [DG, C], BF16, kind="ExternalInput")       # Wout[:, slice].T
    keep = nc.dram_tensor("keep", [128, T], BF16, kind="ExternalInput")  # diag keep (k, q)
    y = nc.dram_tensor("y", [T, C], BF16, kind="ExternalOutput")         # partial output

    NT = T // 512     # 4 t blocks
    NK = C // 128     # 8 contraction chunks
    NTT = T // 128    # 16 t tiles of 128

    with ExitStack() as ctx:
        ctx.enter_context(nc.allow_low_precision(reason="bf16 matmul pipeline"))
        tc = ctx.enter_context(tile.TileContext(nc))
        persist = ctx.enter_context(tc.tile_pool(name="persist", bufs=1))
        psum = ctx.enter_context(tc.tile_pool(name="psum", bufs=2, space="PSUM"))

        # ---- persistent tiles ----
        qT = [persist.tile([128, T], BF16, tag=f"qT{i}", name=f"qT{i}") for i in range(2)]
        kT = [persist.tile([128, T], BF16, tag=f"kT{i}", name=f"kT{i}") for i in range(2)]
        vsb = [persist.tile([128, HPG, KH + 1], BF16, tag=f"v{i}", name=f"v{i}")
               for i in range(NTT)]
        aT2 = [persist.tile([128, T], BF16, tag=f"aT2{g}", name=f"aT2{g}")
               for g in range(2)]
        wo2 = [persist.tile([128, C], BF16, tag=f"wo2{g}", name=f"wo2{g}")
               for g in range(2)]
        keep_sb = persist.tile([128, T], BF16, tag="keep")
        ones_r = persist.tile([65, 64], F32R, tag="ones_r")
        ones_f32 = persist.tile([65, 64], F32, tag="ones_f32")
        xT = persist.tile([128, NK, T], BF16, tag="xT", name="xT")
        wq_sb = persist.tile([128, 2, NK, 128], BF16, tag="wq_sb", name="wq_sb")
        wk_sb = persist.tile([128, 2, NK, 128], BF16, tag="wk_sb", name="wk_sb")
        wv_sb = persist.tile([128, NK, DG], BF16, tag="wv_sb", name="wv_sb")

        # DMA issue order = first-use order. Split the head of the x load so
        # the first matmul group is fed as early as possible.
        nc.sync.dma_start(out=wq_sb[:, 0], in_=wq[:, 0])
        nc.sync.dma_start(out=xT[:, 0:4, 0:512], in_=xt[:, 0:4, 0:512])
        nc.sync.dma_start(out=xT[:, 4:8, 0:512], in_=xt[:, 4:8, 0:512])
        nc.sync.dma_start(out=wq_sb[:, 1], in_=wq[:, 1])
        nc.sync.dma_start(out=wk_sb, in_=wk[:, :, :, :])
        nc.sync.dma_start(out=keep_sb, in_=keep[:, :])
        nc.sync.dma_start(out=wv_sb, in_=wv[:, :, :])
        for n in range(1, NT):
            nc.sync.dma_start(out=xT[:, :, n * 512:(n + 1) * 512],
                              in_=xt[:, :, n * 512:(n + 1) * 512])
        for g in range(2):
            nc.sync.dma_start(out=wo2[g], in_=wo[g * 128:(g + 1) * 128, :])

        nc.vector.memset(ones_f32, 1.0)
        nc.vector.tensor_copy(out=ones_r, in_=ones_f32)
        for tt in range(NTT):
            nc.gpsimd.memset(vsb[tt][:, :, KH:KH + 1], 1.0)

        # ---- projection chains, decomposed into per-matmul filler thunks ----
        def qk_thunks(dst, w_sb, m, n):
            st = {}
            def mm(k):
                if k == 0:
                    st["ps"] = psum.tile([128, 512], F32, tag="pj", name="ps")
                nc.tensor.matmul(
                    st["ps"],
                    w_sb[:, m, k, :],
                    xT[:, k, n * 512:(n + 1) * 512],
                    start=(k == 0), stop=(k == NK - 1),
                )
            def fin():
                nc.vector.tensor_copy(
                    out=dst[m][:, n * 512:(n + 1) * 512], in_=st["ps"])
            return [lambda k=k: mm(k) for k in range(NK)] + [fin]

        def v_thunks(tt):
            st = {}
            def mm(k):
                if k == 0:
                    st["ps"] = psum.tile([128, DG], F32, tag="pj", name="ps")
                nc.tensor.matmul(
                    st["ps"],
                    xT[:, k, tt * 128:(tt + 1) * 128],
                    wv_sb[:, k, :],
                    start=(k == 0), stop=(k == NK - 1),
                )
            def fin():
                nc.vector.tensor_copy(
                    out=vsb[tt][:, :, 0:KH],
                    in_=st["ps"][:].rearrange("p (h d) -> p h d", h=HPG),
                )
            return [lambda k=k: mm(k) for k in range(NK)] + [fin]

        def proj_thunks(n):
            th = []
            for dst, w_sb in ((qT, wq_sb), (kT, wk_sb)):
                for m in range(2):
                    th += qk_thunks(dst, w_sb, m, n)
            for i in range(4):
                th += v_thunks(4 * n + i)
            return th

        def outproj_thunks(qj, last=False):
            th = []
            for i in range(4):
                tt = 4 * qj + i
                st = {}
                def mk(tt, st, no, g, last):
                    def mm():
                        if no == 0 and g == 0:
                            st["yt"] = ytp.tile([128, C], BF16, tag="yt",
                                                name="yt")
                        if g == 0:
                            st["yp"] = psum.tile([128, 512], F32, tag="pj",
                                                 name="yp")
                        nc.tensor.matmul(
                            st["yp"],
                            aT2[g][:, tt * 128:(tt + 1) * 128],
                            wo2[g][:, no * 512:(no + 1) * 512],
                            start=(g == 0), stop=(g == 1),
                        )
                    def fin():
                        sl = slice(no * 512, (no + 1) * 512)
                        if last and no == 1:
                            nc.scalar.activation(
                                out=st["yt"][:, sl], in_=st["yp"], func=COPY)
                        else:
                            nc.vector.tensor_copy(
                                out=st["yt"][:, sl], in_=st["yp"])
                        nc.sync.dma_start(
                            out=y[tt * 128:(tt + 1) * 128, sl],
                            in_=st["yt"][:, sl])
                    return mm, fin
                for no in range(2):
                    for g in range(2):
                        mm, fin = mk(tt, st, no, g, last)
                        th.append(mm)
                        if g == 1:
                            th.append(fin)
            return th

        def attn_unit(qj, hp, fill=None, defer_norm=False):
            kmax = 4 * qj + 4
            acc = [psum.tile([65, 512], F32, tag=f"pv{par}", bufs=1,
                             name=f"pv{par}")
                   for par in range(2)]
            for kt in range(kmax):
                off = 128 * (kt - 4 * qj) if kt >= 4 * qj else 0
                sc2 = psum.tile([128, 1024], F32, tag="sc", name="sc2")
                for par in range(2):
                    nc.tensor.matmul(
                        sc2[:, par * 512 + off:par * 512 + 512],
                        kT[hp][64 * par:64 * par + 64, kt * 128:(kt + 1) * 128],
                        qT[hp][64 * par:64 * par + 64,
                               qj * 512 + off:(qj + 1) * 512],
                        start=True, stop=True,
                    )
                # spread filler thunks evenly over the remaining kt slots so
                # the PE never idles below the HAM activity threshold
                n_take = 0
                if fill:
                    n_take = min(4, -(-len(fill) // (kmax - kt)))
                if n_take:
                    fill.popleft()()
                pt2 = ptp.tile([128, 2, 512], BF16, tag="pt", name="pt2")
                nc.scalar.activation(
                    out=pt2[:, :, off:512],
                    in_=sc2[:].rearrange("p (two n) -> p two n", two=2)[:, :, off:512],
                    func=EXP, scale=0.125)
                if kt >= 4 * qj:
                    for par in range(2):
                        nc.gpsimd.tensor_mul(
                            pt2[:, par, off:off + 128],
                            pt2[:, par, off:off + 128],
                            keep_sb[:, kt * 128:(kt + 1) * 128],
                        )
                for par in range(2):
                    nc.tensor.matmul(
                        acc[par][:, off:512],
                        vsb[kt][:, 2 * hp + par, :],
                        pt2[:, par, off:512],
                        start=(kt == 0), stop=(kt == kmax - 1),
                    )
                for _ in range(min(n_take - 1, len(fill) if fill else 0)):
                    fill.popleft()()
            rinvs = []
            for par in range(2):
                srow = srp.tile([65, 512], F32R, tag="srow", name="srow")
                nc.vector.tensor_copy(out=srow[64:65, :], in_=acc[par][64:65, :])
                rbc = psum.tile([64, 512], F32, tag="pj", name="rbc")
                nc.tensor.matmul(
                    rbc, ones_r[64:65, :], srow[64:65, :], start=True, stop=True)
                rinv = rip.tile([64, 512], F32, tag="rinv", name="rinv")
                nc.vector.reciprocal_approx_fast(out=rinv, in_=rbc)
                rinvs.append(rinv)
                if not defer_norm:
                    nc.vector.tensor_mul(
                        aT2[hp][64 * par:64 * par + 64, qj * 512:(qj + 1) * 512],
                        acc[par][0:64, :],
                        rinv,
                    )
            return acc, rinvs

        from collections import deque

        with tc.tile_pool(name="pts", bufs=8) as ptp, \
             tc.tile_pool(name="srowp", bufs=6) as srp, \
             tc.tile_pool(name="rinvp", bufs=4) as rip, \
             tc.tile_pool(name="ytp", bufs=3) as ytp:
            def burst(th):
                for t in th:
                    t()
            burst(proj_thunks(0))
            # qj 0: drip the head of the n=1 projections through the short
            # attention, burst the rest
            fill = deque(proj_thunks(1))
            attn_unit(0, 0, fill)
            attn_unit(0, 1, fill)
            burst(fill)
            # qj 1..2: drip next block's projections into the kt loop.
            # All output projections are deferred into qj 3's long
            # scalar-paced stretches, which otherwise starve the PE.
            for qj in (1, 2):
                fill = deque(proj_thunks(qj + 1))
                attn_unit(qj, 0, fill)
                attn_unit(qj, 1, fill)
                burst(fill)
            op01 = outproj_thunks(0) + outproj_thunks(1)
            fill = deque(op01[:36])
            attn_unit(3, 0, fill)
            burst(fill)
            fill = deque(op01[36:] + outproj_thunks(2))
            accs, rinvs = attn_unit(3, 1, fill, defer_norm=True)
            burst(fill)
            # final block: per-tile normalization pipelined with its outproj
            op3 = outproj_thunks(3, last=True)
            for i in range(4):
                for par in range(2):
                    nc.vector.tensor_mul(
                        aT2[1][64 * par:64 * par + 64,
                               (12 + i) * 128:(13 + i) * 128],
                        accs[par][0:64, i * 128:(i + 1) * 128],
                        rinvs[par][:, i * 128:(i + 1) * 128],
                    )
                burst(op3[6 * i:6 * (i + 1)])

    _split_excess_waits(nc)
    nc.compile()
    return nc


def _split_excess_waits(nc):
    """Walrus caps most instructions at 1 sync wait. Peel excess waits off
    matmuls (and anything else over the cap) onto PE-engine wait-nops
    inserted immediately before the instruction."""
    for bb in nc.main_func.blocks:
        new_insts = []
        for inst in bb.instructions:
            si = inst.sync_info
            if (si is not None and si.on_wait and len(si.on_wait) > 1
                    and isinstance(inst, mybir.InstMatmult)):
                excess = list(si.on_wait[:-1])
                keep = [si.on_wait[-1]]
                for w in excess:
                    nop = mybir.InstNoOp(
                        name=nc.get_next_instruction_name(), ins=[], outs=[],
                        bass_nofuse=True)
                    nop.engine = inst.engine
                    nop.sync_info = mybir.SyncInfo(on_wait=[w], on_update=[])
                    nc.register_instruction(nop)
                    new_insts.append(nop)
                si.on_wait = keep
            new_insts.append(inst)
        bb.instructions[:] = new_insts


def _host_prep(x, Wq, Wkv, Wout, mask):
    bf16 = ml_dtypes.bfloat16
    x = np.asarray(x, dtype=np.float32)
    Wq = np.asarray(Wq, dtype=np.float32)
    Wkv = np.asarray(Wkv, dtype=np.float32)
    Wout = np.asarray(Wout, dtype=np.float32)
    mask = np.asarray(mask)

    def swiz(a):
        # [C, D] -> partition-major [128, C//128, D], contiguous bf16
        return np.ascontiguousarray(
            a.reshape(C // 128, 128, a.shape[1]).transpose(1, 0, 2)).astype(bf16)

    def swiz_m(a):
        # [C, 256] -> [128, m, C//128, 128], m-major so each half is one
        # contiguous DMA
        return np.ascontiguousarray(
            a.reshape(C // 128, 128, 2, 128).transpose(1, 2, 0, 3)).astype(bf16)

    xT = [swiz(np.ascontiguousarray(x[b].T)) for b in range(B)]
    keep = np.empty((128, T), dtype=np.float32)
    for i in range(T // 128):
        blk = mask[128 * i:128 * (i + 1), 128 * i:128 * (i + 1)]
        keep[:, 128 * i:128 * (i + 1)] = (~blk).T.astype(np.float32)
    keep = keep.astype(bf16)

    in_maps = []
    for core in range(NCORES):
        b, g = core // G, core % G
        sl = slice(DG * g, DG * (g + 1))
        in_maps.append({
            "xt": xT[b],
            "wq": swiz_m(np.ascontiguousarray(Wq[sl, :].T)),
            "wk": swiz_m(np.ascontiguousarray(Wkv[sl, :].T)),
            "wv": swiz(np.ascontiguousarray(Wkv[C + DG * g:C + DG * (g + 1), :].T)),
            "wo": np.ascontiguousarray(Wout[:, sl].T).astype(bf16),
            "keep": keep,
        })
    return in_maps


def _install_ntff_hook():
    import types
    import antenv
    if getattr(antenv, "axon_hooks", None) is not None:
        return
    ah = types.ModuleType("antenv.axon_hooks")
    ah._hook = None
    ah.set_axon_ntff_profile_hook = lambda h: setattr(ah, "_hook", h)
    ah.get_axon_ntff_profile_hook = lambda: ah._hook
    sys.modules["antenv.axon_hooks"] = ah
    antenv.axon_hooks = ah
    if "/root/.axon_site" not in sys.path:
        sys.path.insert(0, "/root/.axon_site")
    from trn_agent_boot.trn_boot import _ntff_profile_via_ctypes
    ah.set_axon_ntff_profile_hook(_ntff_profile_via_ctypes("/opt/axon/libaxon_pjrt.so"))


def _run(inputs, trace=False):
    global _cached_nc
    from concourse.bass_utils import run_bass_kernel_spmd
    if trace:
        _install_ntff_hook()
    if _cached_nc is None:
        _cached_nc = build_nc()
    in_maps = _host_prep(**inputs)
    res = run_bass_kernel_spmd(_cached_nc, in_maps, list(range(NCORES)), trace=trace)
    parts = [np.asarray(res.results[c]["y"], dtype=np.float32) for c in range(NCORES)]
    out = np.stack([
        parts[0] + parts[1] + parts[2] + parts[3],
        parts[4] + parts[5] + parts[6] + parts[7],
    ]).astype(np.float32)
    return out, res


def kernel(x, Wq, Wkv, Wout, mask):
    out, _ = _run(dict(x=x, Wq=Wq, Wkv=Wkv, Wout=Wout, mask=mask))
    return out
